# revision 1
# baseline (speedup 1.0000x reference)
"""DBRX MoE experts kernel for 8 Trainium2 NeuronCores (expert-parallel).

v3: all-bf16 matmuls (same PE rate as fp32r, half the LD_WEIGHTS/DMA bytes),
full-I on-chip h (bf16 h fits SBUF, so w2 streams once and the output is a
single plain write instead of two accumulating passes), deeper PSUM rings,
and DMA ordering that lets the first matmul start early.

  - Host: router matmul + softmax + top-4 + renormalize, gather tokens per
    expert, pre-transpose/re-tile all operands into bf16.
  - Device (SPMD, 8 cores, 2 experts each), per expert:
       GT[i,c] = W1T.T@XT, UT[i,c] = V1T.T@XT   (accumulate over d, PSUM f32)
       HT[i,c] = silu(GT)*UT                    (ACT silu + DVE mult -> bf16)
       YT[d,c] = W2T.T@HT                       (accumulate over all of I)
    Everything is computed transposed ([feature, token]); tokens padded to
    the global max expert group size C.
  - Host: scale rows by gates and scatter-add into the output.
"""
import sys
sys.path.insert(0, "/opt/trn_rl_repo")
import numpy as np
import ml_dtypes

import concourse.bass as bass
import concourse.mybir as mybir
import concourse.tile as tile
import concourse.tile_sem_assignment as _tsa

# Two HWDGE lanes: weights and tokens/outputs travel in parallel. Consumers
# that end up waiting on both lanes get split into single-wait prefixes by
# _split_multi_waits (this walrus build supports one sync-wait per
# instruction).
_tsa.NUM_HWDGE_SEMS = 2

N_CORES = 8
E = 16
E_LOC = 2
D = 2048
I = 4096
TOP_K = 4
NDT = D // 128   # 16 d-tiles
NIT = I // 128   # 32 i-tiles

BF16 = mybir.dt.bfloat16
F32 = mybir.dt.float32
NPBF = ml_dtypes.bfloat16


def _split_multi_waits(nc):
    """Split multi-wait instructions into single-wait EventSemaphore
    prefixes (this walrus build supports one sync-wait per instruction)."""
    ctr = 0
    for f in nc.m.functions:
        for blk in f.blocks:
            insts = list(blk.instructions)
            out = []
            changed = False
            for inst in insts:
                si = inst.sync_info
                if si is not None and si.on_wait is not None and len(si.on_wait) > 1:
                    waits = list(si.on_wait)
                    for w in waits[:-1]:
                        ctr += 1
                        out.append(mybir.InstEventSemaphore(
                            name=f"wsplit_{ctr}",
                            engine=inst.engine,
                            ins=[], outs=[],
                            sync_info=mybir.SyncInfo(on_wait=[w], on_update=[]),
                            bass_nofuse=True,
                        ))
                    inst.sync_info = mybir.SyncInfo(
                        on_wait=[waits[-1]], on_update=list(si.on_update or []))
                    changed = True
                out.append(inst)
            if changed:
                blk.instructions.clear()
                for i2 in out:
                    blk.add_instruction(i2)


def _chunks(n):
    """Split even-length [0, n) into even-sized PSUM chunks (<=512 each)."""
    assert n % 2 == 0
    if n <= 512:
        return [(0, n)]
    k = -(-n // 512)
    sizes = [(n // k) & ~1] * k
    rem, j = n - sum(sizes), 0
    while rem > 0:
        sizes[j] += 2
        rem -= 2
        j = (j + 1) % k
    out, s = [], 0
    for sz in sizes:
        out.append((s, sz))
        s += sz
    return out


def _build_nc(Cs, rep=1):
    """Cs: per-slot token widths (slot 0 = larger expert of the pair)."""
    nc = bass.Bass(target_bir_lowering=False)
    CT = sum(Cs)
    xt_d = nc.dram_tensor("xt", [NDT, 128, CT], BF16, kind="ExternalInput")
    wv1_d = nc.dram_tensor("wv1", [E_LOC, NIT, 128, 2 * NDT * 128], BF16,
                           kind="ExternalInput")
    w2_d = nc.dram_tensor("w2", [E_LOC, NDT, 128, NIT * 128], BF16,
                          kind="ExternalInput")
    yt_d = nc.dram_tensor("yt", [NDT, 128, CT], BF16, kind="ExternalOutput")

    with tile.TileContext(nc) as tc:
        with (
            tc.tile_pool(name="xt", bufs=2) as xt_pool,
            tc.tile_pool(name="ht", bufs=1) as ht_pool,
            tc.tile_pool(name="wg", bufs=3) as wg_pool,
            tc.tile_pool(name="ev", bufs=2) as ev_pool,
            tc.tile_pool(name="ps", bufs=3, space="PSUM") as ps,
        ):
            for rp in range(rep):
                off = 0
                for el in range(E_LOC):
                    _emit_expert(nc, xt_pool, ht_pool, wg_pool, ev_pool, ps,
                                 xt_d, wv1_d, w2_d, yt_d, rp, el, Cs[el], off)
                    off += Cs[el]
    nc.finalize()
    _split_multi_waits(nc)
    return nc


def _emit_expert(nc, xt_pool, ht_pool, wg_pool, ev_pool, ps,
                 xt_d, wv1_d, w2_d, yt_d, rp, el, C, off):
    ch = _chunks(C)
    tb = f"{rp}_{el}"
    # --- token tiles (resident for the whole expert; bufs=2 so the next
    # expert's tokens stream in while this expert computes) ---
    wv0 = wg_pool.tile([128, 2, NDT, 128], BF16, tag="wg",
                       name=f"wv_{tb}_0")
    nc.sync.dma_start(
        wv0[:], wv1_d[el, 0].rearrange("p (w t i) -> p w t i", w=2, t=NDT))
    xts = xt_pool.tile([128, NDT, C], BF16, tag="xt", name=f"xt_{tb}")
    # two descriptors: dt 0-1 land fast so the first matmuls start early
    nc.sync.dma_start(xts[:, 0:2, :],
                      xt_d[0:2, :, off:off + C].rearrange("t p c -> p t c"))
    nc.sync.dma_start(xts[:, 2:8, :],
                      xt_d[2:8, :, off:off + C].rearrange("t p c -> p t c"))
    nc.sync.dma_start(xts[:, 8:, :],
                      xt_d[8:, :, off:off + C].rearrange("t p c -> p t c"))
    hts = ht_pool.tile([128, NIT, C], BF16, tag="ht", name=f"ht_{tb}")
    # --- stage 1+2: HT = silu(W1T.T@X) * (V1T.T@X), full I ---
    for it in range(NIT):
        if it == 0:
            wv = wv0
        else:
            wv = wg_pool.tile([128, 2, NDT, 128], BF16, tag="wg",
                              name=f"wv_{tb}_{it}")
            nc.sync.dma_start(
                wv[:], wv1_d[el, it].rearrange("p (w t i) -> p w t i",
                                               w=2, t=NDT))
        gs = [ps.tile([128, cn], F32, tag=f"pq{ci}", name=f"g{ci}_{tb}_{it}",
                      bufs=(3 if ci < 2 else 2))
              for ci, (c0, cn) in enumerate(ch)]
        for dt in range(NDT):
            for ci, (c0, cn) in enumerate(ch):
                nc.tensor.matmul(gs[ci][:], wv[:, 0, dt, :],
                                 xts[:, dt, c0:c0 + cn],
                                 start=(dt == 0), stop=(dt == NDT - 1))
        ss = []
        for ci, (c0, cn) in enumerate(ch):
            s = ev_pool.tile([128, cn], BF16, tag=f"hs{ci}",
                             name=f"hs{ci}_{tb}_{it}")
            nc.scalar.activation(s[:], gs[ci][:],
                                 mybir.ActivationFunctionType.Silu)
            ss.append(s)
        us = [ps.tile([128, cn], F32, tag=f"pq{ci}", name=f"u{ci}_{tb}_{it}",
                      bufs=(3 if ci < 2 else 2))
              for ci, (c0, cn) in enumerate(ch)]
        for dt in range(NDT):
            for ci, (c0, cn) in enumerate(ch):
                nc.tensor.matmul(us[ci][:], wv[:, 1, dt, :],
                                 xts[:, dt, c0:c0 + cn],
                                 start=(dt == 0), stop=(dt == NDT - 1))
        for ci, (c0, cn) in enumerate(ch):
            nc.vector.tensor_tensor(
                out=hts[:, it, c0:c0 + cn], in0=us[ci][:],
                in1=ss[ci][:], op=mybir.AluOpType.mult)
    # --- stage 3: YT[dt] = W2T.T @ HT (contract all of I) ---
    for dt in range(NDT):
        w2 = wg_pool.tile([128, NIT, 128], BF16, tag="w2",
                          name=f"w2_{tb}_{dt}")
        nc.sync.dma_start(
            w2[:], w2_d[el, dt].rearrange("p (u i) -> p u i", u=NIT))
        ys = [ps.tile([128, cn], F32, tag=f"pq{ci}", name=f"y{ci}_{tb}_{dt}",
                      bufs=(3 if ci < 2 else 2))
              for ci, (c0, cn) in enumerate(ch)]
        for it in range(NIT):
            for ci, (c0, cn) in enumerate(ch):
                nc.tensor.matmul(ys[ci][:], w2[:, it, :],
                                 hts[:, it, c0:c0 + cn],
                                 start=(it == 0), stop=(it == NIT - 1))
        yo = ev_pool.tile([128, C], BF16, tag="yo", name=f"yo_{tb}_{dt}")
        last = (el == E_LOC - 1 and dt == NDT - 1)
        for ci, (c0, cn) in enumerate(ch):
            nc.scalar.activation(yo[:, c0:c0 + cn], ys[ci][:],
                                 mybir.ActivationFunctionType.Copy)
            if last:
                nc.sync.dma_start(yt_d[dt, :, off + c0:off + c0 + cn],
                                  yo[:, c0:c0 + cn])
        if not last:
            nc.sync.dma_start(yt_d[dt, :, off:off + C], yo[:])


def _prepare(hidden_states, router_w, ws, w2s, rep=1):
    hs = np.ascontiguousarray(hidden_states, dtype=np.float32)
    rw = np.ascontiguousarray(router_w, dtype=np.float32)
    ws = np.asarray(ws, dtype=np.float32)
    w2s = np.asarray(w2s, dtype=np.float32)
    T, D_ = hs.shape
    assert (D_, ws.shape[0], ws.shape[1], w2s.shape[1], w2s.shape[2]) == \
        (D, E, 2 * I, D, I), "kernel compiled for DBRX 16x(2048->4096) shapes"

    # ---- routing on host (softmax -> top-4 -> renormalize) ----
    logits = hs @ rw.T
    m = logits.max(axis=-1, keepdims=True)
    p = np.exp(logits - m)
    p /= p.sum(axis=-1, keepdims=True)
    topk_idx = np.argpartition(-p, TOP_K - 1, axis=-1)[:, :TOP_K]
    topk_val = np.take_along_axis(p, topk_idx, axis=-1)
    gates_w = topk_val / topk_val.sum(axis=-1, keepdims=True)

    tok_idx, tok_gate = [None] * E, [None] * E
    flat_e = topk_idx.ravel()
    flat_g = gates_w.ravel()
    flat_t = np.repeat(np.arange(T), TOP_K)
    order = np.argsort(flat_e, kind="stable")
    se, st, sg = flat_e[order], flat_t[order], flat_g[order]
    bounds = np.searchsorted(se, np.arange(E + 1))
    for e in range(E):
        tok_idx[e] = st[bounds[e]:bounds[e + 1]]
        tok_gate[e] = sg[bounds[e]:bounds[e + 1]]

    # ---- pair large experts with small ones to minimize padded width ----
    sizes = np.array([len(t) for t in tok_idx])
    order = np.argsort(-sizes, kind="stable")
    assign = [[int(order[c]), int(order[2 * N_CORES - 1 - c])]
              for c in range(N_CORES)]
    Cs = []
    for el in range(E_LOC):
        w = max(4, int(max(sizes[assign[c][el]] for c in range(N_CORES))))
        Cs.append(w + w % 2)
    CT = sum(Cs)
    offs = [0, Cs[0]]

    # ---- build per-core inputs (bf16) ----
    in_maps = []
    for c in range(N_CORES):
        xt = np.zeros((NDT, 128, CT), dtype=NPBF)
        wv1 = np.empty((E_LOC, NIT, 128, 2 * NDT * 128), dtype=NPBF)
        w2 = np.empty((E_LOC, NDT, 128, NIT * 128), dtype=NPBF)
        for el in range(E_LOC):
            e = assign[c][el]
            xg = hs[tok_idx[e]]                          # [n_e, D]
            xt[:, :, offs[el]:offs[el] + len(tok_idx[e])] = \
                xg.T.reshape(NDT, 128, -1).astype(NPBF)
            w1 = ws[e, :I, :]
            v1 = ws[e, I:, :]
            # wv1[el, it, p, (w, dt, ii)] = {w1,v1}[it*128+ii, dt*128+p]
            wv = np.stack([w1, v1]).reshape(2, NIT, 128, NDT, 128)
            wv = wv.transpose(1, 4, 0, 3, 2)             # [it, p, w, dt, ii]
            wv1[el] = np.ascontiguousarray(wv).reshape(
                NIT, 128, -1).astype(NPBF)
            # w2[el, dt, p, (it, ii)] = w2s[e, dt*128+ii, it*128+p]
            w2e = w2s[e].reshape(NDT, 128, NIT, 128)     # [dt, ii, it, p]
            w2[el] = np.ascontiguousarray(
                w2e.transpose(0, 3, 2, 1)).reshape(NDT, 128, -1).astype(NPBF)
        in_maps.append({"xt": xt, "wv1": wv1, "w2": w2})

    def combine(results):
        out = np.zeros((T, D), dtype=np.float32)
        for c in range(N_CORES):
            yt = results[c]["yt"]                        # [NDT, 128, CT]
            yf = yt.reshape(D, CT).astype(np.float32)
            for el in range(E_LOC):
                e = assign[c][el]
                n_e = len(tok_idx[e])
                if n_e == 0:
                    continue
                y = yf[:, offs[el]:offs[el] + n_e].T     # [n_e, D]
                out[tok_idx[e]] += tok_gate[e][:, None].astype(np.float32) * y
        return out

    nc = _build_nc(Cs, rep=rep)
    return {"nc": nc, "in_maps": in_maps, "combine": combine, "C": Cs}


def kernel(hidden_states, router_w, ws, w2s):
    from concourse.bass_utils import run_bass_kernel_spmd
    prep = _prepare(hidden_states, router_w, ws, w2s)
    res = run_bass_kernel_spmd(prep["nc"], prep["in_maps"],
                               core_ids=list(range(N_CORES)))
    return prep["combine"](res.results)



# revision 5
# speedup vs baseline: 1.1553x; 1.1553x over previous
"""DBRX MoE experts kernel for 8 Trainium2 NeuronCores (expert-parallel).

v4: mixed-precision. Each expert's tokens are split by gate weight:
high-gate tokens run the gate/up matmuls (stage 1+2) in bf16 as before;
low-gate tokens (gate <= THETA, ~26% of columns carrying ~6% of the
squared gate mass) run stage 1+2 in fp8-e4m3 with DoubleRow matmuls
(256-deep contraction pairs, 2 MACs/cell/cycle). h stays bf16 (at a
folded scale) so stage 3 is plain bf16 for every column, and the final
copy de-scales the fp8 columns. Simulated end-to-end rel-err 1.55e-2
vs the 2e-2 gate.

  - Host: router matmul + softmax + top-4 + renormalize, gather tokens
    per expert, split by gate, pre-transpose/re-tile operands (bf16 +
    scaled fp8 copies of w1/v1/x for the low-gate groups).
  - Device (SPMD, 8 cores, 2 experts each), per expert:
       bf16 cols: GT/UT = W1T.T@XT / V1T.T@XT   (PSUM f32, NDT matmuls)
       fp8  cols: GT/UT via DoubleRow pairs      (NDT/2 matmuls)
       HT = silu(GT)*UT -> bf16 (fp8 cols keep a sv*SX scale folded in)
       YT[d] = W2T.T@HT  (bf16, all cols; fp8 cols de-scaled in the copy)
  - Host: scale rows by gates and scatter-add into the output.
"""
import sys
sys.path.insert(0, "/opt/trn_rl_repo")
import numpy as np
import ml_dtypes

import concourse.bass as bass
import concourse.mybir as mybir
import concourse.tile as tile
import concourse.tile_sem_assignment as _tsa

# Two HWDGE lanes: weights and tokens/outputs travel in parallel.
_tsa.NUM_HWDGE_SEMS = 2

N_CORES = 8
E = 16
E_LOC = 2
D = 2048
I = 4096
TOP_K = 4
NDT = D // 128   # 16 d-tiles
NDP = NDT // 2   # 8 d-tile pairs (DoubleRow)
NIT = I // 128   # 32 i-tiles

# Cap on the fraction of total squared gate mass routed through the fp8
# path. 0.078 simulates to ~1.6e-2 end-to-end rel err (gate is 2e-2).
SHARE_CAP = 0.078

BF16 = mybir.dt.bfloat16
F32 = mybir.dt.float32
FP8 = mybir.dt.float8e4
NPBF = ml_dtypes.bfloat16
NP8 = ml_dtypes.float8_e4m3
DR = mybir.MatmulPerfMode.DoubleRow


def _split_multi_waits(nc):
    """Split multi-wait instructions into single-wait EventSemaphore
    prefixes (this walrus build supports one sync-wait per instruction)."""
    ctr = 0
    for f in nc.m.functions:
        for blk in f.blocks:
            insts = list(blk.instructions)
            out = []
            changed = False
            for inst in insts:
                si = inst.sync_info
                if si is not None and si.on_wait is not None and len(si.on_wait) > 1:
                    waits = list(si.on_wait)
                    for w in waits[:-1]:
                        ctr += 1
                        out.append(mybir.InstEventSemaphore(
                            name=f"wsplit_{ctr}",
                            engine=inst.engine,
                            ins=[], outs=[],
                            sync_info=mybir.SyncInfo(on_wait=[w], on_update=[]),
                            bass_nofuse=True,
                        ))
                    inst.sync_info = mybir.SyncInfo(
                        on_wait=[waits[-1]], on_update=list(si.on_update or []))
                    changed = True
                out.append(inst)
            if changed:
                blk.instructions.clear()
                for i2 in out:
                    blk.add_instruction(i2)


def _chunks(n):
    """Split even-length [0, n) into even-sized PSUM chunks (<=512 each)."""
    assert n % 2 == 0
    if n <= 512:
        return [(0, n)]
    k = -(-n // 512)
    sizes = [(n // k) & ~1] * k
    rem, j = n - sum(sizes), 0
    while rem > 0:
        sizes[j] += 2
        rem -= 2
        j = (j + 1) % k
    out, s = [], 0
    for sz in sizes:
        out.append((s, sz))
        s += sz
    return out


def _build_nc(CBs, CFs, rep=1):
    """CBs/CFs: per-slot bf16/fp8 token widths (same for all cores)."""
    nc = bass.Bass(target_bir_lowering=False)
    CBT = sum(CBs)
    CFT = sum(CFs)
    CT = CBT + CFT
    xt_d = nc.dram_tensor("xt", [NDT, 128, CBT], BF16, kind="ExternalInput")
    x8_d = nc.dram_tensor("x8", [NDT, 128, CFT], FP8, kind="ExternalInput")
    wv1_d = nc.dram_tensor("wv1", [E_LOC, NIT, 128, 2 * NDT * 128], BF16,
                           kind="ExternalInput")
    # fp8 stationary pairs: per (el, it, partition): (w, dpair, slot, i)
    wv8_d = nc.dram_tensor("wv8", [E_LOC, NIT, 128, 2 * NDT * 128], FP8,
                           kind="ExternalInput")
    w2_d = nc.dram_tensor("w2", [E_LOC, NDT, 128, NIT * 128], BF16,
                          kind="ExternalInput")
    scl_d = nc.dram_tensor("scl", [128, E_LOC, 2], F32, kind="ExternalInput")
    yt_d = nc.dram_tensor("yt", [NDT, 128, CT], BF16, kind="ExternalOutput")

    with tile.TileContext(nc) as tc:
        with (
            tc.tile_pool(name="xt", bufs=2) as xt_pool,
            tc.tile_pool(name="ht", bufs=1) as ht_pool,
            tc.tile_pool(name="wg", bufs=3) as wg_pool,
            tc.tile_pool(name="ev", bufs=2) as ev_pool,
            tc.tile_pool(name="ps", bufs=2, space="PSUM") as ps,
        ):
            scl = ev_pool.tile([128, E_LOC, 2], F32, tag="scl", name="scl",
                               bufs=1)
            nc.sync.dma_start(scl[:], scl_d[:])
            for rp in range(rep):
                offb = 0
                offf = 0
                for el in range(E_LOC):
                    _emit_expert(nc, xt_pool, ht_pool, wg_pool, ev_pool, ps,
                                 xt_d, x8_d, wv1_d, wv8_d, w2_d, yt_d, scl,
                                 rp, el, CBs, CFs, offb, offf)
                    offb += CBs[el]
                    offf += CFs[el]
    nc.finalize()
    _split_multi_waits(nc)
    return nc


def _emit_expert(nc, xt_pool, ht_pool, wg_pool, ev_pool, ps,
                 xt_d, x8_d, wv1_d, wv8_d, w2_d, yt_d, scl,
                 rp, el, CBs, CFs, offb, offf):
    CB, CF = CBs[el], CFs[el]
    CBT, CFT = sum(CBs), sum(CFs)
    C = CB + CF
    # hts / yt column layout: [b0 | f0 | b1 | f1]
    hoff = (offb + offf)
    ch = _chunks(CB)
    tb = f"{rp}_{el}"
    # --- token tiles (resident for the whole expert) ---
    wv0 = wg_pool.tile([128, 2, NDT, 128], BF16, tag="wg", name=f"wv_{tb}_0")
    nc.sync.dma_start(
        wv0[:], wv1_d[el, 0].rearrange("p (w t i) -> p w t i", w=2, t=NDT))
    xts = xt_pool.tile([128, NDT, CB], BF16, tag="xt", name=f"xt_{tb}")
    nc.sync.dma_start(xts[:, 0:2, :],
                      xt_d[0:2, :, offb:offb + CB].rearrange("t p c -> p t c"))
    nc.sync.dma_start(xts[:, 2:8, :],
                      xt_d[2:8, :, offb:offb + CB].rearrange("t p c -> p t c"))
    nc.sync.dma_start(xts[:, 8:, :],
                      xt_d[8:, :, offb:offb + CB].rearrange("t p c -> p t c"))
    x8s = xt_pool.tile([128, NDT, CF], FP8, tag="x8", name=f"x8_{tb}")
    nc.sync.dma_start(x8s[:],
                      x8_d[:, :, offf:offf + CF].rearrange("t p c -> p t c"))
    hts = ht_pool.tile([128, NIT, C], BF16, tag="ht", name=f"ht_{tb}")
    # --- stage 1+2: HT = silu(W1T.T@X) * (V1T.T@X), full I ---
    for it in range(NIT):
        if it == 0:
            wv = wv0
        else:
            wv = wg_pool.tile([128, 2, NDT, 128], BF16, tag="wg",
                              name=f"wv_{tb}_{it}")
            nc.sync.dma_start(
                wv[:], wv1_d[el, it].rearrange("p (w t i) -> p w t i",
                                               w=2, t=NDT))
        wv8 = wg_pool.tile([128, 2, NDP, 2, 128], FP8, tag="wg8",
                           name=f"wv8_{tb}_{it}")
        nc.sync.dma_start(
            wv8[:], wv8_d[el, it].rearrange("p (w u s i) -> p w u s i",
                                            w=2, u=NDP, s=2))
        # bf16 group
        gs = [ps.tile([128, cn], F32, tag=f"pq{ci}", name=f"g{ci}_{tb}_{it}")
              for ci, (c0, cn) in enumerate(ch)]
        for dt in range(NDT):
            for ci, (c0, cn) in enumerate(ch):
                nc.tensor.matmul(gs[ci][:], wv[:, 0, dt, :],
                                 xts[:, dt, c0:c0 + cn],
                                 start=(dt == 0), stop=(dt == NDT - 1))
        # fp8 group (DoubleRow pairs)
        gf = ps.tile([128, CF], F32, tag="pf", name=f"gf_{tb}_{it}")
        for u in range(NDP):
            nc.tensor.matmul(gf[:], wv8[:, 0, u], x8s[:, 2 * u:2 * u + 2, :],
                             start=(u == 0), stop=(u == NDP - 1),
                             perf_mode=DR)
        ss = []
        for ci, (c0, cn) in enumerate(ch):
            s = ev_pool.tile([128, cn], BF16, tag=f"hs{ci}",
                             name=f"hs{ci}_{tb}_{it}")
            nc.scalar.activation(s[:], gs[ci][:],
                                 mybir.ActivationFunctionType.Silu)
            ss.append(s)
        sf = ev_pool.tile([128, CF], BF16, tag="hsf", name=f"hsf_{tb}_{it}")
        nc.scalar.activation(sf[:], gf[:], mybir.ActivationFunctionType.Silu,
                             scale=scl[:, el, 0:1])
        us = [ps.tile([128, cn], F32, tag=f"pq{ci}", name=f"u{ci}_{tb}_{it}")
              for ci, (c0, cn) in enumerate(ch)]
        for dt in range(NDT):
            for ci, (c0, cn) in enumerate(ch):
                nc.tensor.matmul(us[ci][:], wv[:, 1, dt, :],
                                 xts[:, dt, c0:c0 + cn],
                                 start=(dt == 0), stop=(dt == NDT - 1))
        uf = ps.tile([128, CF], F32, tag="pf", name=f"uf_{tb}_{it}")
        for u in range(NDP):
            nc.tensor.matmul(uf[:], wv8[:, 1, u], x8s[:, 2 * u:2 * u + 2, :],
                             start=(u == 0), stop=(u == NDP - 1),
                             perf_mode=DR)
        for ci, (c0, cn) in enumerate(ch):
            nc.vector.tensor_tensor(
                out=hts[:, it, c0:c0 + cn], in0=us[ci][:],
                in1=ss[ci][:], op=mybir.AluOpType.mult)
        nc.vector.tensor_tensor(
            out=hts[:, it, CB:CB + CF], in0=uf[:],
            in1=sf[:], op=mybir.AluOpType.mult)
    # --- stage 3: YT[dt] = W2T.T @ HT (contract all of I), all cols bf16 ---
    ch3 = ch + [(CB, CF)]
    for dt in range(NDT):
        w2 = wg_pool.tile([128, NIT, 128], BF16, tag="w2",
                          name=f"w2_{tb}_{dt}")
        nc.sync.dma_start(
            w2[:], w2_d[el, dt].rearrange("p (u i) -> p u i", u=NIT))
        ys = [ps.tile([128, cn], F32,
                      tag=("pf" if ci == len(ch3) - 1 else f"pq{ci}"),
                      name=f"y{ci}_{tb}_{dt}")
              for ci, (c0, cn) in enumerate(ch3)]
        for it in range(NIT):
            for ci, (c0, cn) in enumerate(ch3):
                nc.tensor.matmul(ys[ci][:], w2[:, it, :],
                                 hts[:, it, c0:c0 + cn],
                                 start=(it == 0), stop=(it == NIT - 1))
        yo = ev_pool.tile([128, C], BF16, tag="yo", name=f"yo_{tb}_{dt}")
        last = (el == E_LOC - 1 and dt == NDT - 1)
        for ci, (c0, cn) in enumerate(ch3):
            if ci == len(ch3) - 1:
                nc.scalar.activation(yo[:, c0:c0 + cn], ys[ci][:],
                                     mybir.ActivationFunctionType.Copy,
                                     scale=scl[:, el, 1:2])
            else:
                nc.scalar.activation(yo[:, c0:c0 + cn], ys[ci][:],
                                     mybir.ActivationFunctionType.Copy)
            if last:
                nc.sync.dma_start(yt_d[dt, :, hoff + c0:hoff + c0 + cn],
                                  yo[:, c0:c0 + cn])
        if not last:
            nc.sync.dma_start(yt_d[dt, :, hoff:hoff + C], yo[:])


def _even(n, lo=4):
    n = max(lo, int(n))
    return n + (n % 2)


def _prepare(hidden_states, router_w, ws, w2s, rep=1):
    hs = np.ascontiguousarray(hidden_states, dtype=np.float32)
    rw = np.ascontiguousarray(router_w, dtype=np.float32)
    ws = np.asarray(ws, dtype=np.float32)
    w2s = np.asarray(w2s, dtype=np.float32)
    T, D_ = hs.shape
    assert (D_, ws.shape[0], ws.shape[1], w2s.shape[1], w2s.shape[2]) == \
        (D, E, 2 * I, D, I), "kernel compiled for DBRX 16x(2048->4096) shapes"

    # ---- routing on host (softmax -> top-4 -> renormalize) ----
    logits = hs @ rw.T
    m = logits.max(axis=-1, keepdims=True)
    p = np.exp(logits - m)
    p /= p.sum(axis=-1, keepdims=True)
    topk_idx = np.argpartition(-p, TOP_K - 1, axis=-1)[:, :TOP_K]
    topk_val = np.take_along_axis(p, topk_idx, axis=-1)
    gates_w = topk_val / topk_val.sum(axis=-1, keepdims=True)

    tok_idx, tok_gate = [None] * E, [None] * E
    flat_e = topk_idx.ravel()
    flat_g = gates_w.ravel()
    flat_t = np.repeat(np.arange(T), TOP_K)
    order = np.argsort(flat_e, kind="stable")
    se, st, sg = flat_e[order], flat_t[order], flat_g[order]
    bounds = np.searchsorted(se, np.arange(E + 1))
    for e in range(E):
        ti = st[bounds[e]:bounds[e + 1]]
        tg = sg[bounds[e]:bounds[e + 1]]
        # sort tokens by gate descending: [bf16 (big gates), fp8 (small)]
        o = np.argsort(-tg, kind="stable")
        tok_idx[e] = ti[o]
        tok_gate[e] = tg[o]

    # ---- pair large experts with small ones; slot width = per-slot max ----
    sizes = np.array([len(t) for t in tok_idx])
    order = np.argsort(-sizes, kind="stable")
    assign = [[int(order[c]), int(order[2 * N_CORES - 1 - c])]
              for c in range(N_CORES)]
    slot_exp = [[assign[c][el] for c in range(N_CORES)]
                for el in range(E_LOC)]

    # ---- split: uniform bf16 count NB per slot (zero bf16 padding); an
    # expert's smallest-gate (C_e - NB) tokens take the fp8 path. NB totals
    # are set by the squared-gate-share error budget. ----
    g2tot = float(sum((tok_gate[e] ** 2).sum() for e in range(E)))
    tail2 = [np.cumsum(tok_gate[e][::-1].astype(np.float64) ** 2)
             for e in range(E)]  # tail2[e][k-1] = sum of k smallest gates^2

    def share_of(nbs):
        s = 0.0
        for el in range(E_LOC):
            for e in slot_exp[el]:
                k = len(tok_gate[e]) - nbs[el]
                if k > 0:
                    s += tail2[e][k - 1]
        return s / g2tot

    def nbs_for(nbtot):
        d = int(np.mean([sizes[e] for e in slot_exp[0]])
                - np.mean([sizes[e] for e in slot_exp[1]]))
        nb0 = min((nbtot + d) // 2, min(sizes[e] for e in slot_exp[0]))
        nb1 = min(nbtot - nb0, min(sizes[e] for e in slot_exp[1]))
        nb0 = nbtot - nb1
        return [nb0 & ~1, nb1 & ~1]

    lo, hi = 2 * E_LOC, int(min(sizes[e] for e in slot_exp[0])
                            + min(sizes[e] for e in slot_exp[1]))
    while lo < hi:  # find max NBtot with share <= cap (share grows as NB drops)
        mid = (lo + hi) // 2
        if share_of(nbs_for(mid)) <= SHARE_CAP:
            hi = mid
        else:
            lo = mid + 1
    NBs = nbs_for(lo)
    nb = [0] * E
    for el in range(E_LOC):
        for e in slot_exp[el]:
            nb[e] = min(NBs[el], len(tok_gate[e]))
    nf = [len(tok_gate[e]) - nb[e] for e in range(E)]

    CBs, CFs = [], []
    for el in range(E_LOC):
        CBs.append(_even(max(nb[assign[c][el]] for c in range(N_CORES))))
        cf = _even(max(nf[assign[c][el]] for c in range(N_CORES)), lo=16)
        CFs.append(-(-cf // 16) * 16)
    assert max(CFs) <= 512, CFs
    CBT, CFT = sum(CBs), sum(CFs)
    offb = [0, CBs[0]]
    offf = [0, CFs[0]]
    hoff = [0, CBs[0] + CFs[0]]

    SX = float(224.0 / np.abs(hs).max())

    # ---- build per-core inputs ----
    in_maps = []
    for c in range(N_CORES):
        xt = np.zeros((NDT, 128, CBT), dtype=NPBF)
        x8 = np.zeros((NDT, 128, CFT), dtype=NP8)
        wv1 = np.empty((E_LOC, NIT, 128, 2 * NDT * 128), dtype=NPBF)
        wv8 = np.empty((E_LOC, NIT, 128, 2 * NDT * 128), dtype=NP8)
        w2 = np.empty((E_LOC, NDT, 128, NIT * 128), dtype=NPBF)
        scl = np.empty((128, E_LOC, 2), dtype=np.float32)
        for el in range(E_LOC):
            e = assign[c][el]
            nbe, nfe = nb[e], nf[e]
            xb = hs[tok_idx[e][:nbe]]                    # [nbe, D]
            xf = hs[tok_idx[e][nbe:]]                    # [nfe, D]
            xt[:, :, offb[el]:offb[el] + nbe] = \
                xb.T.reshape(NDT, 128, -1).astype(NPBF)
            x8[:, :, offf[el]:offf[el] + nfe] = np.clip(
                xf.T.reshape(NDT, 128, -1) * SX, -240, 240).astype(NP8)
            w1 = ws[e, :I, :]
            v1 = ws[e, I:, :]
            sw = float(224.0 / np.abs(w1).max())
            sv = float(224.0 / np.abs(v1).max())
            scl[:, el, 0] = 1.0 / (sw * SX)
            scl[:, el, 1] = 1.0 / (sv * SX)
            # wv1[el, it, p, (w, dt, ii)] = {w1,v1}[it*128+ii, dt*128+p]
            wv = np.stack([w1, v1]).reshape(2, NIT, 128, NDT, 128)
            wv = wv.transpose(1, 4, 0, 3, 2)             # [it, p, w, dt, ii]
            wv1[el] = np.ascontiguousarray(wv).reshape(
                NIT, 128, -1).astype(NPBF)
            # wv8[el, it, p, (w, u, s, ii)] = {w1*sw, v1*sv}[it*128+ii,
            #                                               (2u+s)*128+p]
            wq = np.stack([np.clip(w1 * sw, -240, 240),
                           np.clip(v1 * sv, -240, 240)])
            wq = wq.reshape(2, NIT, 128, NDP, 2, 128)    # [w,it,ii,u,s,p]
            wq = wq.transpose(1, 5, 0, 3, 4, 2)          # [it,p,w,u,s,ii]
            wv8[el] = np.ascontiguousarray(wq).reshape(
                NIT, 128, -1).astype(NP8)
            # w2[el, dt, p, (it, ii)] = w2s[e, dt*128+ii, it*128+p]
            w2e = w2s[e].reshape(NDT, 128, NIT, 128)     # [dt, ii, it, p]
            w2[el] = np.ascontiguousarray(
                w2e.transpose(0, 3, 2, 1)).reshape(NDT, 128, -1).astype(NPBF)
        in_maps.append({"xt": xt, "x8": x8, "wv1": wv1, "wv8": wv8,
                        "w2": w2, "scl": scl})

    def combine(results):
        out = np.zeros((T, D), dtype=np.float32)
        for c in range(N_CORES):
            yt = results[c]["yt"]                        # [NDT, 128, CT]
            yf = yt.reshape(D, CBT + CFT).astype(np.float32)
            for el in range(E_LOC):
                e = assign[c][el]
                n_e = len(tok_idx[e])
                if n_e == 0:
                    continue
                nbe = nb[e]
                y = np.empty((n_e, D), dtype=np.float32)
                y[:nbe] = yf[:, hoff[el]:hoff[el] + nbe].T
                y[nbe:] = yf[:, hoff[el] + CBs[el]:
                             hoff[el] + CBs[el] + (n_e - nbe)].T
                out[tok_idx[e]] += \
                    tok_gate[e][:, None].astype(np.float32) * y
        return out

    nc = _build_nc(CBs, CFs, rep=rep)
    return {"nc": nc, "in_maps": in_maps, "combine": combine,
            "C": (CBs, CFs)}


def kernel(hidden_states, router_w, ws, w2s):
    from concourse.bass_utils import run_bass_kernel_spmd
    prep = _prepare(hidden_states, router_w, ws, w2s)
    res = run_bass_kernel_spmd(prep["nc"], prep["in_maps"],
                               core_ids=list(range(N_CORES)))
    return prep["combine"](res.results)


# revision 9
# speedup vs baseline: 1.1660x; 1.0093x over previous
"""DBRX MoE experts kernel for 8 Trainium2 NeuronCores (expert-parallel).

v4: mixed-precision. Each expert's tokens are split by gate weight:
high-gate tokens run the gate/up matmuls (stage 1+2) in bf16 as before;
low-gate tokens (gate <= THETA, ~26% of columns carrying ~6% of the
squared gate mass) run stage 1+2 in fp8-e4m3 with DoubleRow matmuls
(256-deep contraction pairs, 2 MACs/cell/cycle). h stays bf16 (at a
folded scale) so stage 3 is plain bf16 for every column, and the final
copy de-scales the fp8 columns. Simulated end-to-end rel-err 1.55e-2
vs the 2e-2 gate.

  - Host: router matmul + softmax + top-4 + renormalize, gather tokens
    per expert, split by gate, pre-transpose/re-tile operands (bf16 +
    scaled fp8 copies of w1/v1/x for the low-gate groups).
  - Device (SPMD, 8 cores, 2 experts each), per expert:
       bf16 cols: GT/UT = W1T.T@XT / V1T.T@XT   (PSUM f32, NDT matmuls)
       fp8  cols: GT/UT via DoubleRow pairs      (NDT/2 matmuls)
       HT = silu(GT)*UT -> bf16 (fp8 cols keep a sv*SX scale folded in)
       YT[d] = W2T.T@HT  (bf16, all cols; fp8 cols de-scaled in the copy)
  - Host: scale rows by gates and scatter-add into the output.
"""
import sys
sys.path.insert(0, "/opt/trn_rl_repo")
import numpy as np
import ml_dtypes

import concourse.bass as bass
import concourse.mybir as mybir
import concourse.tile as tile
import concourse.tile_sem_assignment as _tsa

# Four HWDGE lanes (round-robin): weights, tokens and outputs spread
# across queues so startup fills and the output drain run in parallel.
_tsa.NUM_HWDGE_SEMS = 4

N_CORES = 8
E = 16
E_LOC = 2
D = 2048
I = 4096
TOP_K = 4
NDT = D // 128   # 16 d-tiles
NDP = NDT // 2   # 8 d-tile pairs (DoubleRow)
NIT = I // 128   # 32 i-tiles

# Cap on the fraction of total squared gate mass routed through the fp8
# path. 0.078 measured 1.57e-2 end-to-end rel err on HW (gate is 2e-2);
# error scales ~sqrt(share) above the bf16 floor.
SHARE_CAP = 0.088

BF16 = mybir.dt.bfloat16
F32 = mybir.dt.float32
FP8 = mybir.dt.float8e4
NPBF = ml_dtypes.bfloat16
NP8 = ml_dtypes.float8_e4m3
DR = mybir.MatmulPerfMode.DoubleRow


def _split_multi_waits(nc):
    """Split multi-wait instructions into single-wait EventSemaphore
    prefixes (this walrus build supports one sync-wait per instruction)."""
    ctr = 0
    for f in nc.m.functions:
        for blk in f.blocks:
            insts = list(blk.instructions)
            out = []
            changed = False
            for inst in insts:
                si = inst.sync_info
                if si is not None and si.on_wait is not None and len(si.on_wait) > 1:
                    waits = list(si.on_wait)
                    for w in waits[:-1]:
                        ctr += 1
                        out.append(mybir.InstEventSemaphore(
                            name=f"wsplit_{ctr}",
                            engine=inst.engine,
                            ins=[], outs=[],
                            sync_info=mybir.SyncInfo(on_wait=[w], on_update=[]),
                            bass_nofuse=True,
                        ))
                    inst.sync_info = mybir.SyncInfo(
                        on_wait=[waits[-1]], on_update=list(si.on_update or []))
                    changed = True
                out.append(inst)
            if changed:
                blk.instructions.clear()
                for i2 in out:
                    blk.add_instruction(i2)


def _chunks(n):
    """Split even-length [0, n) into even-sized PSUM chunks (<=512 each)."""
    assert n % 2 == 0
    if n <= 512:
        return [(0, n)]
    k = -(-n // 512)
    sizes = [(n // k) & ~1] * k
    rem, j = n - sum(sizes), 0
    while rem > 0:
        sizes[j] += 2
        rem -= 2
        j = (j + 1) % k
    out, s = [], 0
    for sz in sizes:
        out.append((s, sz))
        s += sz
    return out


def _build_nc(CBs, CFs, rep=1):
    """CBs/CFs: per-slot bf16/fp8 token widths (same for all cores)."""
    nc = bass.Bass(target_bir_lowering=False)
    CBT = sum(CBs)
    CFT = sum(CFs)
    CT = CBT + CFT
    xt_d = nc.dram_tensor("xt", [NDT, 128, CBT], BF16, kind="ExternalInput")
    x8_d = nc.dram_tensor("x8", [NDT, 128, CFT], FP8, kind="ExternalInput")
    wv1_d = nc.dram_tensor("wv1", [E_LOC, NIT, 128, 2 * NDT * 128], BF16,
                           kind="ExternalInput")
    # fp8 stationary pairs: per (el, it, partition): (w, dpair, slot, i)
    wv8_d = nc.dram_tensor("wv8", [E_LOC, NIT, 128, 2 * NDT * 128], FP8,
                           kind="ExternalInput")
    w2_d = nc.dram_tensor("w2", [E_LOC, NDT, 128, NIT * 128], BF16,
                          kind="ExternalInput")
    scl_d = nc.dram_tensor("scl", [128, E_LOC, 2], F32, kind="ExternalInput")
    yt_d = nc.dram_tensor("yt", [NDT, 128, CT], BF16, kind="ExternalOutput")

    with tile.TileContext(nc) as tc:
        with (
            tc.tile_pool(name="xt", bufs=2) as xt_pool,
            tc.tile_pool(name="ht", bufs=1) as ht_pool,
            tc.tile_pool(name="wg", bufs=3) as wg_pool,
            tc.tile_pool(name="ev", bufs=2) as ev_pool,
            tc.tile_pool(name="ps", bufs=2, space="PSUM") as ps,
        ):
            scl = ev_pool.tile([128, E_LOC, 2], F32, tag="scl", name="scl",
                               bufs=1)
            nc.sync.dma_start(scl[:], scl_d[:])
            for rp in range(rep):
                offb = 0
                offf = 0
                for el in range(E_LOC):
                    _emit_expert(nc, xt_pool, ht_pool, wg_pool, ev_pool, ps,
                                 xt_d, x8_d, wv1_d, wv8_d, w2_d, yt_d, scl,
                                 rp, el, CBs, CFs, offb, offf)
                    offb += CBs[el]
                    offf += CFs[el]
    nc.finalize()
    _split_multi_waits(nc)
    return nc


def _emit_expert(nc, xt_pool, ht_pool, wg_pool, ev_pool, ps,
                 xt_d, x8_d, wv1_d, wv8_d, w2_d, yt_d, scl,
                 rp, el, CBs, CFs, offb, offf):
    CB, CF = CBs[el], CFs[el]
    CBT, CFT = sum(CBs), sum(CFs)
    C = CB + CF
    # hts / yt column layout: [b0 | f0 | b1 | f1]
    hoff = (offb + offf)
    ch = _chunks(CB)
    tb = f"{rp}_{el}"
    # --- token tiles (resident for the whole expert) ---
    wv0 = wg_pool.tile([128, 2, NDT, 128], BF16, tag="wg", name=f"wv_{tb}_0")
    wv0r = wv1_d[el, 0].rearrange("p (w t i) -> p w t i", w=2, t=NDT)
    nc.sync.dma_start(wv0[:, :, 0:2, :], wv0r[:, :, 0:2, :])
    xts = xt_pool.tile([128, NDT, CB], BF16, tag="xt", name=f"xt_{tb}")
    nc.sync.dma_start(xts[:, 0:2, :],
                      xt_d[0:2, :, offb:offb + CB].rearrange("t p c -> p t c"))
    nc.sync.dma_start(wv0[:, :, 2:, :], wv0r[:, :, 2:, :])
    nc.sync.dma_start(xts[:, 2:8, :],
                      xt_d[2:8, :, offb:offb + CB].rearrange("t p c -> p t c"))
    nc.sync.dma_start(xts[:, 8:, :],
                      xt_d[8:, :, offb:offb + CB].rearrange("t p c -> p t c"))
    x8s = xt_pool.tile([128, NDT, CF], FP8, tag="x8", name=f"x8_{tb}")
    nc.sync.dma_start(x8s[:],
                      x8_d[:, :, offf:offf + CF].rearrange("t p c -> p t c"))
    hts = ht_pool.tile([128, NIT, C], BF16, tag="ht", name=f"ht_{tb}")
    # --- stage 1+2: HT = silu(W1T.T@X) * (V1T.T@X), full I ---
    for it in range(NIT):
        if it == 0:
            wv = wv0
        else:
            wv = wg_pool.tile([128, 2, NDT, 128], BF16, tag="wg",
                              name=f"wv_{tb}_{it}")
            nc.sync.dma_start(
                wv[:], wv1_d[el, it].rearrange("p (w t i) -> p w t i",
                                               w=2, t=NDT))
        wv8 = wg_pool.tile([128, 2, NDP, 2, 128], FP8, tag="wg8",
                           name=f"wv8_{tb}_{it}")
        nc.sync.dma_start(
            wv8[:], wv8_d[el, it].rearrange("p (w u s i) -> p w u s i",
                                            w=2, u=NDP, s=2))
        # bf16 group
        gs = [ps.tile([128, cn], F32, tag=f"pq{ci}", name=f"g{ci}_{tb}_{it}")
              for ci, (c0, cn) in enumerate(ch)]
        for dt in range(NDT):
            for ci, (c0, cn) in enumerate(ch):
                nc.tensor.matmul(gs[ci][:], wv[:, 0, dt, :],
                                 xts[:, dt, c0:c0 + cn],
                                 start=(dt == 0), stop=(dt == NDT - 1))
        # fp8 group (DoubleRow pairs)
        gf = ps.tile([128, CF], F32, tag="pf", name=f"gf_{tb}_{it}")
        for u in range(NDP):
            nc.tensor.matmul(gf[:], wv8[:, 0, u], x8s[:, 2 * u:2 * u + 2, :],
                             start=(u == 0), stop=(u == NDP - 1),
                             perf_mode=DR)
        ss = []
        for ci, (c0, cn) in enumerate(ch):
            s = ev_pool.tile([128, cn], BF16, tag=f"hs{ci}",
                             name=f"hs{ci}_{tb}_{it}")
            nc.scalar.activation(s[:], gs[ci][:],
                                 mybir.ActivationFunctionType.Silu)
            ss.append(s)
        sf = ev_pool.tile([128, CF], BF16, tag="hsf", name=f"hsf_{tb}_{it}")
        nc.scalar.activation(sf[:], gf[:], mybir.ActivationFunctionType.Silu,
                             scale=scl[:, el, 0:1])
        us = [ps.tile([128, cn], F32, tag=f"pq{ci}", name=f"u{ci}_{tb}_{it}")
              for ci, (c0, cn) in enumerate(ch)]
        for dt in range(NDT):
            for ci, (c0, cn) in enumerate(ch):
                nc.tensor.matmul(us[ci][:], wv[:, 1, dt, :],
                                 xts[:, dt, c0:c0 + cn],
                                 start=(dt == 0), stop=(dt == NDT - 1))
        uf = ps.tile([128, CF], F32, tag="pf", name=f"uf_{tb}_{it}")
        for u in range(NDP):
            nc.tensor.matmul(uf[:], wv8[:, 1, u], x8s[:, 2 * u:2 * u + 2, :],
                             start=(u == 0), stop=(u == NDP - 1),
                             perf_mode=DR)
        for ci, (c0, cn) in enumerate(ch):
            nc.vector.tensor_tensor(
                out=hts[:, it, c0:c0 + cn], in0=us[ci][:],
                in1=ss[ci][:], op=mybir.AluOpType.mult)
        nc.vector.tensor_tensor(
            out=hts[:, it, CB:CB + CF], in0=uf[:],
            in1=sf[:], op=mybir.AluOpType.mult)
    # --- stage 3: YT[dt] = W2T.T @ HT (contract all of I), all cols bf16 ---
    ch3 = ch + [(CB, CF)]
    for dt in range(NDT):
        w2 = wg_pool.tile([128, NIT, 128], BF16, tag="w2",
                          name=f"w2_{tb}_{dt}")
        nc.sync.dma_start(
            w2[:], w2_d[el, dt].rearrange("p (u i) -> p u i", u=NIT))
        ys = [ps.tile([128, cn], F32,
                      tag=("pf" if ci == len(ch3) - 1 else f"pq{ci}"),
                      name=f"y{ci}_{tb}_{dt}")
              for ci, (c0, cn) in enumerate(ch3)]
        for it in range(NIT):
            for ci, (c0, cn) in enumerate(ch3):
                nc.tensor.matmul(ys[ci][:], w2[:, it, :],
                                 hts[:, it, c0:c0 + cn],
                                 start=(it == 0), stop=(it == NIT - 1))
        yo = ev_pool.tile([128, C], BF16, tag="yo", name=f"yo_{tb}_{dt}")
        fine = (el == E_LOC - 1)
        for ci, (c0, cn) in enumerate(ch3):
            if ci == len(ch3) - 1:
                nc.scalar.activation(yo[:, c0:c0 + cn], ys[ci][:],
                                     mybir.ActivationFunctionType.Copy,
                                     scale=scl[:, el, 1:2])
            else:
                nc.scalar.activation(yo[:, c0:c0 + cn], ys[ci][:],
                                     mybir.ActivationFunctionType.Copy)
            if fine:
                nc.sync.dma_start(yt_d[dt, :, hoff + c0:hoff + c0 + cn],
                                  yo[:, c0:c0 + cn])
        if not fine:
            nc.sync.dma_start(yt_d[dt, :, hoff:hoff + C], yo[:])


def _even(n, lo=4):
    n = max(lo, int(n))
    return n + (n % 2)


def _prepare(hidden_states, router_w, ws, w2s, rep=1):
    hs = np.ascontiguousarray(hidden_states, dtype=np.float32)
    rw = np.ascontiguousarray(router_w, dtype=np.float32)
    ws = np.asarray(ws, dtype=np.float32)
    w2s = np.asarray(w2s, dtype=np.float32)
    T, D_ = hs.shape
    assert (D_, ws.shape[0], ws.shape[1], w2s.shape[1], w2s.shape[2]) == \
        (D, E, 2 * I, D, I), "kernel compiled for DBRX 16x(2048->4096) shapes"

    # ---- routing on host (softmax -> top-4 -> renormalize) ----
    logits = hs @ rw.T
    m = logits.max(axis=-1, keepdims=True)
    p = np.exp(logits - m)
    p /= p.sum(axis=-1, keepdims=True)
    topk_idx = np.argpartition(-p, TOP_K - 1, axis=-1)[:, :TOP_K]
    topk_val = np.take_along_axis(p, topk_idx, axis=-1)
    gates_w = topk_val / topk_val.sum(axis=-1, keepdims=True)

    tok_idx, tok_gate = [None] * E, [None] * E
    flat_e = topk_idx.ravel()
    flat_g = gates_w.ravel()
    flat_t = np.repeat(np.arange(T), TOP_K)
    order = np.argsort(flat_e, kind="stable")
    se, st, sg = flat_e[order], flat_t[order], flat_g[order]
    bounds = np.searchsorted(se, np.arange(E + 1))
    for e in range(E):
        ti = st[bounds[e]:bounds[e + 1]]
        tg = sg[bounds[e]:bounds[e + 1]]
        # sort tokens by gate descending: [bf16 (big gates), fp8 (small)]
        o = np.argsort(-tg, kind="stable")
        tok_idx[e] = ti[o]
        tok_gate[e] = tg[o]

    # ---- pair large experts with small ones; slot width = per-slot max ----
    sizes = np.array([len(t) for t in tok_idx])
    order = np.argsort(-sizes, kind="stable")
    assign = [[int(order[c]), int(order[2 * N_CORES - 1 - c])]
              for c in range(N_CORES)]
    slot_exp = [[assign[c][el] for c in range(N_CORES)]
                for el in range(E_LOC)]

    # ---- split: uniform bf16 count NB per slot (zero bf16 padding); an
    # expert's smallest-gate (C_e - NB) tokens take the fp8 path. NB totals
    # are set by the squared-gate-share error budget. ----
    g2tot = float(sum((tok_gate[e] ** 2).sum() for e in range(E)))
    tail2 = [np.cumsum(tok_gate[e][::-1].astype(np.float64) ** 2)
             for e in range(E)]  # tail2[e][k-1] = sum of k smallest gates^2

    def share_of(nbs):
        s = 0.0
        for el in range(E_LOC):
            for e in slot_exp[el]:
                k = len(tok_gate[e]) - nbs[el]
                if k > 0:
                    s += tail2[e][k - 1]
        return s / g2tot

    def nbs_for(nbtot):
        d = int(np.mean([sizes[e] for e in slot_exp[0]])
                - np.mean([sizes[e] for e in slot_exp[1]]))
        nb0 = min((nbtot + d) // 2, min(sizes[e] for e in slot_exp[0]))
        nb1 = min(nbtot - nb0, min(sizes[e] for e in slot_exp[1]))
        nb0 = nbtot - nb1
        return [nb0 & ~1, nb1 & ~1]

    lo, hi = 2 * E_LOC, int(min(sizes[e] for e in slot_exp[0])
                            + min(sizes[e] for e in slot_exp[1]))
    while lo < hi:  # find max NBtot with share <= cap (share grows as NB drops)
        mid = (lo + hi) // 2
        if share_of(nbs_for(mid)) <= SHARE_CAP:
            hi = mid
        else:
            lo = mid + 1
    NBs = nbs_for(lo)
    nb = [0] * E
    for el in range(E_LOC):
        for e in slot_exp[el]:
            nb[e] = min(NBs[el], len(tok_gate[e]))
    nf = [len(tok_gate[e]) - nb[e] for e in range(E)]

    CBs, CFs = [], []
    for el in range(E_LOC):
        CBs.append(_even(max(nb[assign[c][el]] for c in range(N_CORES))))
        cf = _even(max(nf[assign[c][el]] for c in range(N_CORES)), lo=16)
        CFs.append(-(-cf // 16) * 16)
    assert max(CFs) <= 512, CFs
    CBT, CFT = sum(CBs), sum(CFs)
    offb = [0, CBs[0]]
    offf = [0, CFs[0]]
    hoff = [0, CBs[0] + CFs[0]]

    SX = float(224.0 / np.abs(hs).max())

    # ---- build per-core inputs ----
    in_maps = []
    for c in range(N_CORES):
        xt = np.zeros((NDT, 128, CBT), dtype=NPBF)
        x8 = np.zeros((NDT, 128, CFT), dtype=NP8)
        wv1 = np.empty((E_LOC, NIT, 128, 2 * NDT * 128), dtype=NPBF)
        wv8 = np.empty((E_LOC, NIT, 128, 2 * NDT * 128), dtype=NP8)
        w2 = np.empty((E_LOC, NDT, 128, NIT * 128), dtype=NPBF)
        scl = np.empty((128, E_LOC, 2), dtype=np.float32)
        for el in range(E_LOC):
            e = assign[c][el]
            nbe, nfe = nb[e], nf[e]
            xb = hs[tok_idx[e][:nbe]]                    # [nbe, D]
            xf = hs[tok_idx[e][nbe:]]                    # [nfe, D]
            xt[:, :, offb[el]:offb[el] + nbe] = \
                xb.T.reshape(NDT, 128, -1).astype(NPBF)
            x8[:, :, offf[el]:offf[el] + nfe] = np.clip(
                xf.T.reshape(NDT, 128, -1) * SX, -240, 240).astype(NP8)
            w1 = ws[e, :I, :]
            v1 = ws[e, I:, :]
            sw = float(224.0 / np.abs(w1).max())
            sv = float(224.0 / np.abs(v1).max())
            scl[:, el, 0] = 1.0 / (sw * SX)
            scl[:, el, 1] = 1.0 / (sv * SX)
            # wv1[el, it, p, (w, dt, ii)] = {w1,v1}[it*128+ii, dt*128+p]
            wv = np.stack([w1, v1]).reshape(2, NIT, 128, NDT, 128)
            wv = wv.transpose(1, 4, 0, 3, 2)             # [it, p, w, dt, ii]
            wv1[el] = np.ascontiguousarray(wv).reshape(
                NIT, 128, -1).astype(NPBF)
            # wv8[el, it, p, (w, u, s, ii)] = {w1*sw, v1*sv}[it*128+ii,
            #                                               (2u+s)*128+p]
            wq = np.stack([np.clip(w1 * sw, -240, 240),
                           np.clip(v1 * sv, -240, 240)])
            wq = wq.reshape(2, NIT, 128, NDP, 2, 128)    # [w,it,ii,u,s,p]
            wq = wq.transpose(1, 5, 0, 3, 4, 2)          # [it,p,w,u,s,ii]
            wv8[el] = np.ascontiguousarray(wq).reshape(
                NIT, 128, -1).astype(NP8)
            # w2[el, dt, p, (it, ii)] = w2s[e, dt*128+ii, it*128+p]
            w2e = w2s[e].reshape(NDT, 128, NIT, 128)     # [dt, ii, it, p]
            w2[el] = np.ascontiguousarray(
                w2e.transpose(0, 3, 2, 1)).reshape(NDT, 128, -1).astype(NPBF)
        in_maps.append({"xt": xt, "x8": x8, "wv1": wv1, "wv8": wv8,
                        "w2": w2, "scl": scl})

    def combine(results):
        out = np.zeros((T, D), dtype=np.float32)
        for c in range(N_CORES):
            yt = results[c]["yt"]                        # [NDT, 128, CT]
            yf = yt.reshape(D, CBT + CFT).astype(np.float32)
            for el in range(E_LOC):
                e = assign[c][el]
                n_e = len(tok_idx[e])
                if n_e == 0:
                    continue
                nbe = nb[e]
                y = np.empty((n_e, D), dtype=np.float32)
                y[:nbe] = yf[:, hoff[el]:hoff[el] + nbe].T
                y[nbe:] = yf[:, hoff[el] + CBs[el]:
                             hoff[el] + CBs[el] + (n_e - nbe)].T
                out[tok_idx[e]] += \
                    tok_gate[e][:, None].astype(np.float32) * y
        return out

    nc = _build_nc(CBs, CFs, rep=rep)
    return {"nc": nc, "in_maps": in_maps, "combine": combine,
            "C": (CBs, CFs)}


def kernel(hidden_states, router_w, ws, w2s):
    from concourse.bass_utils import run_bass_kernel_spmd
    prep = _prepare(hidden_states, router_w, ws, w2s)
    res = run_bass_kernel_spmd(prep["nc"], prep["in_maps"],
                               core_ids=list(range(N_CORES)))
    return prep["combine"](res.results)


# revision 14
# speedup vs baseline: 1.1734x; 1.0064x over previous
"""DBRX MoE experts kernel for 8 Trainium2 NeuronCores (expert-parallel).

v4: mixed-precision. Each expert's tokens are split by gate weight:
high-gate tokens run the gate/up matmuls (stage 1+2) in bf16 as before;
low-gate tokens (gate <= THETA, ~26% of columns carrying ~6% of the
squared gate mass) run stage 1+2 in fp8-e4m3 with DoubleRow matmuls
(256-deep contraction pairs, 2 MACs/cell/cycle). h stays bf16 (at a
folded scale) so stage 3 is plain bf16 for every column, and the final
copy de-scales the fp8 columns. Simulated end-to-end rel-err 1.55e-2
vs the 2e-2 gate.

  - Host: router matmul + softmax + top-4 + renormalize, gather tokens
    per expert, split by gate, pre-transpose/re-tile operands (bf16 +
    scaled fp8 copies of w1/v1/x for the low-gate groups).
  - Device (SPMD, 8 cores, 2 experts each), per expert:
       bf16 cols: GT/UT = W1T.T@XT / V1T.T@XT   (PSUM f32, NDT matmuls)
       fp8  cols: GT/UT via DoubleRow pairs      (NDT/2 matmuls)
       HT = silu(GT)*UT -> bf16 (fp8 cols keep a sv*SX scale folded in)
       YT[d] = W2T.T@HT  (bf16, all cols; fp8 cols de-scaled in the copy)
  - Host: scale rows by gates and scatter-add into the output.
"""
import sys
sys.path.insert(0, "/opt/trn_rl_repo")
import numpy as np
import ml_dtypes

import concourse.bass as bass
import concourse.mybir as mybir
import concourse.tile as tile
import concourse.tile_sem_assignment as _tsa

# Four HWDGE lanes (round-robin): weights, tokens and outputs spread
# across queues so startup fills and the output drain run in parallel.
_tsa.NUM_HWDGE_SEMS = 4

N_CORES = 8
E = 16
E_LOC = 2
D = 2048
I = 4096
TOP_K = 4
NDT = D // 128   # 16 d-tiles
NDP = NDT // 2   # 8 d-tile pairs (DoubleRow)
NIT = I // 128   # 32 i-tiles

# Cap on the fraction of total squared gate mass routed through the fp8
# path. 0.078 measured 1.57e-2 and 0.088 measured 1.66e-2 end-to-end rel
# err on HW (gate is 2e-2); error scales ~sqrt(share) above the bf16
# floor, so 0.098 predicts ~1.75e-2.
SHARE_CAP = 0.098

BF16 = mybir.dt.bfloat16
F32 = mybir.dt.float32
FP8 = mybir.dt.float8e4
NPBF = ml_dtypes.bfloat16
NP8 = ml_dtypes.float8_e4m3
DR = mybir.MatmulPerfMode.DoubleRow


def _split_multi_waits(nc):
    """Split multi-wait instructions into single-wait EventSemaphore
    prefixes (this walrus build supports one sync-wait per instruction)."""
    ctr = 0
    for f in nc.m.functions:
        for blk in f.blocks:
            insts = list(blk.instructions)
            out = []
            changed = False
            for inst in insts:
                si = inst.sync_info
                if si is not None and si.on_wait is not None and len(si.on_wait) > 1:
                    waits = list(si.on_wait)
                    for w in waits[:-1]:
                        ctr += 1
                        out.append(mybir.InstEventSemaphore(
                            name=f"wsplit_{ctr}",
                            engine=inst.engine,
                            ins=[], outs=[],
                            sync_info=mybir.SyncInfo(on_wait=[w], on_update=[]),
                            bass_nofuse=True,
                        ))
                    inst.sync_info = mybir.SyncInfo(
                        on_wait=[waits[-1]], on_update=list(si.on_update or []))
                    changed = True
                out.append(inst)
            if changed:
                blk.instructions.clear()
                for i2 in out:
                    blk.add_instruction(i2)


def _chunks(n):
    """Split even-length [0, n) into even-sized PSUM chunks (<=512 each)."""
    assert n % 2 == 0
    if n <= 512:
        return [(0, n)]
    k = -(-n // 512)
    sizes = [(n // k) & ~1] * k
    rem, j = n - sum(sizes), 0
    while rem > 0:
        sizes[j] += 2
        rem -= 2
        j = (j + 1) % k
    out, s = [], 0
    for sz in sizes:
        out.append((s, sz))
        s += sz
    return out


def _build_nc(CBs, CFs, rep=1):
    """CBs/CFs: per-slot bf16/fp8 token widths (same for all cores)."""
    nc = bass.Bass(target_bir_lowering=False)
    CBT = sum(CBs)
    CFT = sum(CFs)
    CT = CBT + CFT
    xt_d = nc.dram_tensor("xt", [NDT, 128, CBT], BF16, kind="ExternalInput")
    x8_d = nc.dram_tensor("x8", [NDT, 128, CFT], FP8, kind="ExternalInput")
    wv1_d = nc.dram_tensor("wv1", [E_LOC, NIT, 128, 2 * NDT * 128], BF16,
                           kind="ExternalInput")
    # fp8 stationary pairs: per (el, it, partition): (w, dpair, slot, i)
    wv8_d = nc.dram_tensor("wv8", [E_LOC, NIT, 128, 2 * NDT * 128], FP8,
                           kind="ExternalInput")
    w2_d = nc.dram_tensor("w2", [E_LOC, NDT, 128, NIT * 128], BF16,
                          kind="ExternalInput")
    scl_d = nc.dram_tensor("scl", [128, E_LOC, 2], F32, kind="ExternalInput")
    yt_d = nc.dram_tensor("yt", [NDT, 128, CT], BF16, kind="ExternalOutput")

    with tile.TileContext(nc) as tc:
        with (
            tc.tile_pool(name="xt", bufs=2) as xt_pool,
            tc.tile_pool(name="ht", bufs=1) as ht_pool,
            tc.tile_pool(name="wg", bufs=3) as wg_pool,
            tc.tile_pool(name="ev", bufs=2) as ev_pool,
            tc.tile_pool(name="ps", bufs=2, space="PSUM") as ps,
        ):
            # PE warm-up burst on uninitialized data: ~5us of matmuls with
            # no DMA dependency trip the HAM clock-gate to 2.4GHz while the
            # first real tiles are still in flight.
            warm_w = ev_pool.tile([128, 128], BF16, tag="warmw",
                                  name="warm_w", bufs=1)
            warm_x = ev_pool.tile([128, 512], BF16, tag="warmx",
                                  name="warm_x", bufs=1)
            warm_p = ps.tile([128, 512], F32, tag="warm", name="warm_p",
                             bufs=1)
            nc.any.memset(warm_w[:], 0)
            nc.any.memset(warm_x[:], 0)
            for k in range(14):
                nc.tensor.matmul(warm_p[:], warm_w[:], warm_x[:],
                                 start=(k == 0), stop=(k == 13))
            scl = ev_pool.tile([128, E_LOC, 2], F32, tag="scl", name="scl",
                               bufs=1)
            nc.sync.dma_start(scl[:], scl_d[:])
            for rp in range(rep):
                offb = 0
                offf = 0
                for el in range(E_LOC):
                    _emit_expert(nc, xt_pool, ht_pool, wg_pool, ev_pool, ps,
                                 xt_d, x8_d, wv1_d, wv8_d, w2_d, yt_d, scl,
                                 rp, el, CBs, CFs, offb, offf)
                    offb += CBs[el]
                    offf += CFs[el]
    nc.finalize()
    _split_multi_waits(nc)
    return nc


def _emit_expert(nc, xt_pool, ht_pool, wg_pool, ev_pool, ps,
                 xt_d, x8_d, wv1_d, wv8_d, w2_d, yt_d, scl,
                 rp, el, CBs, CFs, offb, offf):
    CB, CF = CBs[el], CFs[el]
    CBT, CFT = sum(CBs), sum(CFs)
    C = CB + CF
    # hts / yt column layout: [b0 | f0 | b1 | f1]
    hoff = (offb + offf)
    ch = _chunks(CB)
    tb = f"{rp}_{el}"
    # --- token tiles (resident for the whole expert) ---
    wv0 = wg_pool.tile([128, 2, NDT, 128], BF16, tag="wg", name=f"wv_{tb}_0")
    wv0r = wv1_d[el, 0].rearrange("p (w t i) -> p w t i", w=2, t=NDT)
    xts = xt_pool.tile([128, NDT, CB], BF16, tag="xt", name=f"xt_{tb}")
    xtr = xt_d[:, :, offb:offb + CB]
    if el == 0 and rp == 0:
        # fine-grained first pieces: the first matmuls wait on ~320KB only
        nc.sync.dma_start(wv0[:, :, 0:2, :], wv0r[:, :, 0:2, :])
        nc.sync.dma_start(xts[:, 0:1, :],
                          xtr[0:1].rearrange("t p c -> p t c"))
        nc.sync.dma_start(xts[:, 1:2, :],
                          xtr[1:2].rearrange("t p c -> p t c"))
        nc.sync.dma_start(wv0[:, :, 2:, :], wv0r[:, :, 2:, :])
    else:
        nc.sync.dma_start(wv0[:, :, 0:2, :], wv0r[:, :, 0:2, :])
        nc.sync.dma_start(xts[:, 0:2, :],
                          xtr[0:2].rearrange("t p c -> p t c"))
        nc.sync.dma_start(wv0[:, :, 2:, :], wv0r[:, :, 2:, :])
    x8s = xt_pool.tile([128, NDT, CF], FP8, tag="x8", name=f"x8_{tb}")
    nc.sync.dma_start(x8s[:],
                      x8_d[:, :, offf:offf + CF].rearrange("t p c -> p t c"))
    nc.sync.dma_start(xts[:, 2:8, :], xtr[2:8].rearrange("t p c -> p t c"))
    nc.sync.dma_start(xts[:, 8:, :], xtr[8:].rearrange("t p c -> p t c"))
    hts = ht_pool.tile([128, NIT, C], BF16, tag="ht", name=f"ht_{tb}")
    # --- stage 1+2: HT = silu(W1T.T@X) * (V1T.T@X), full I ---
    for it in range(NIT):
        if it == 0:
            wv = wv0
        else:
            wv = wg_pool.tile([128, 2, NDT, 128], BF16, tag="wg",
                              name=f"wv_{tb}_{it}")
            nc.sync.dma_start(
                wv[:], wv1_d[el, it].rearrange("p (w t i) -> p w t i",
                                               w=2, t=NDT))
        wv8 = wg_pool.tile([128, 2, NDP, 2, 128], FP8, tag="wg8",
                           name=f"wv8_{tb}_{it}")
        nc.sync.dma_start(
            wv8[:], wv8_d[el, it].rearrange("p (w u s i) -> p w u s i",
                                            w=2, u=NDP, s=2))
        # bf16 group
        gs = [ps.tile([128, cn], F32, tag=f"pq{ci}", name=f"g{ci}_{tb}_{it}")
              for ci, (c0, cn) in enumerate(ch)]
        for dt in range(NDT):
            for ci, (c0, cn) in enumerate(ch):
                nc.tensor.matmul(gs[ci][:], wv[:, 0, dt, :],
                                 xts[:, dt, c0:c0 + cn],
                                 start=(dt == 0), stop=(dt == NDT - 1))
        # fp8 group (DoubleRow pairs)
        gf = ps.tile([128, CF], F32, tag="pf", name=f"gf_{tb}_{it}")
        for u in range(NDP):
            nc.tensor.matmul(gf[:], wv8[:, 0, u], x8s[:, 2 * u:2 * u + 2, :],
                             start=(u == 0), stop=(u == NDP - 1),
                             perf_mode=DR)
        ss = []
        for ci, (c0, cn) in enumerate(ch):
            s = ev_pool.tile([128, cn], BF16, tag=f"hs{ci}",
                             name=f"hs{ci}_{tb}_{it}")
            nc.scalar.activation(s[:], gs[ci][:],
                                 mybir.ActivationFunctionType.Silu)
            ss.append(s)
        sf = ev_pool.tile([128, CF], BF16, tag="hsf", name=f"hsf_{tb}_{it}")
        nc.scalar.activation(sf[:], gf[:], mybir.ActivationFunctionType.Silu,
                             scale=scl[:, el, 0:1])
        us = [ps.tile([128, cn], F32, tag=f"pq{ci}", name=f"u{ci}_{tb}_{it}")
              for ci, (c0, cn) in enumerate(ch)]
        for dt in range(NDT):
            for ci, (c0, cn) in enumerate(ch):
                nc.tensor.matmul(us[ci][:], wv[:, 1, dt, :],
                                 xts[:, dt, c0:c0 + cn],
                                 start=(dt == 0), stop=(dt == NDT - 1))
        uf = ps.tile([128, CF], F32, tag="pf", name=f"uf_{tb}_{it}")
        for u in range(NDP):
            nc.tensor.matmul(uf[:], wv8[:, 1, u], x8s[:, 2 * u:2 * u + 2, :],
                             start=(u == 0), stop=(u == NDP - 1),
                             perf_mode=DR)
        for ci, (c0, cn) in enumerate(ch):
            nc.vector.tensor_tensor(
                out=hts[:, it, c0:c0 + cn], in0=us[ci][:],
                in1=ss[ci][:], op=mybir.AluOpType.mult)
        nc.vector.tensor_tensor(
            out=hts[:, it, CB:CB + CF], in0=uf[:],
            in1=sf[:], op=mybir.AluOpType.mult)
    # --- stage 3: YT[dt] = W2T.T @ HT (contract all of I), all cols bf16 ---
    ch3 = ch + [(CB, CF)]
    for dt in range(NDT):
        w2 = wg_pool.tile([128, NIT, 128], BF16, tag="w2",
                          name=f"w2_{tb}_{dt}")
        nc.sync.dma_start(
            w2[:], w2_d[el, dt].rearrange("p (u i) -> p u i", u=NIT))
        ys = [ps.tile([128, cn], F32,
                      tag=("pf" if ci == len(ch3) - 1 else f"pq{ci}"),
                      name=f"y{ci}_{tb}_{dt}")
              for ci, (c0, cn) in enumerate(ch3)]
        for it in range(NIT):
            for ci, (c0, cn) in enumerate(ch3):
                nc.tensor.matmul(ys[ci][:], w2[:, it, :],
                                 hts[:, it, c0:c0 + cn],
                                 start=(it == 0), stop=(it == NIT - 1))
        yo = ev_pool.tile([128, C], BF16, tag="yo", name=f"yo_{tb}_{dt}")
        fine = (el == E_LOC - 1)
        for ci, (c0, cn) in enumerate(ch3):
            if ci == len(ch3) - 1:
                nc.scalar.activation(yo[:, c0:c0 + cn], ys[ci][:],
                                     mybir.ActivationFunctionType.Copy,
                                     scale=scl[:, el, 1:2])
            else:
                nc.scalar.activation(yo[:, c0:c0 + cn], ys[ci][:],
                                     mybir.ActivationFunctionType.Copy)
            if fine:
                nc.sync.dma_start(yt_d[dt, :, hoff + c0:hoff + c0 + cn],
                                  yo[:, c0:c0 + cn])
        if not fine:
            nc.sync.dma_start(yt_d[dt, :, hoff:hoff + C], yo[:])


def _even(n, lo=4):
    n = max(lo, int(n))
    return n + (n % 2)


def _prepare(hidden_states, router_w, ws, w2s, rep=1):
    hs = np.ascontiguousarray(hidden_states, dtype=np.float32)
    rw = np.ascontiguousarray(router_w, dtype=np.float32)
    ws = np.asarray(ws, dtype=np.float32)
    w2s = np.asarray(w2s, dtype=np.float32)
    T, D_ = hs.shape
    assert (D_, ws.shape[0], ws.shape[1], w2s.shape[1], w2s.shape[2]) == \
        (D, E, 2 * I, D, I), "kernel compiled for DBRX 16x(2048->4096) shapes"

    # ---- routing on host (softmax -> top-4 -> renormalize) ----
    logits = hs @ rw.T
    m = logits.max(axis=-1, keepdims=True)
    p = np.exp(logits - m)
    p /= p.sum(axis=-1, keepdims=True)
    topk_idx = np.argpartition(-p, TOP_K - 1, axis=-1)[:, :TOP_K]
    topk_val = np.take_along_axis(p, topk_idx, axis=-1)
    gates_w = topk_val / topk_val.sum(axis=-1, keepdims=True)

    tok_idx, tok_gate = [None] * E, [None] * E
    flat_e = topk_idx.ravel()
    flat_g = gates_w.ravel()
    flat_t = np.repeat(np.arange(T), TOP_K)
    order = np.argsort(flat_e, kind="stable")
    se, st, sg = flat_e[order], flat_t[order], flat_g[order]
    bounds = np.searchsorted(se, np.arange(E + 1))
    for e in range(E):
        ti = st[bounds[e]:bounds[e + 1]]
        tg = sg[bounds[e]:bounds[e + 1]]
        # sort tokens by gate descending: [bf16 (big gates), fp8 (small)]
        o = np.argsort(-tg, kind="stable")
        tok_idx[e] = ti[o]
        tok_gate[e] = tg[o]

    # ---- pair large experts with small ones; slot width = per-slot max ----
    sizes = np.array([len(t) for t in tok_idx])
    order = np.argsort(-sizes, kind="stable")
    assign = [[int(order[c]), int(order[2 * N_CORES - 1 - c])]
              for c in range(N_CORES)]
    slot_exp = [[assign[c][el] for c in range(N_CORES)]
                for el in range(E_LOC)]

    # ---- split: uniform bf16 count NB per slot (zero bf16 padding); an
    # expert's smallest-gate (C_e - NB) tokens take the fp8 path. NB totals
    # are set by the squared-gate-share error budget. ----
    g2tot = float(sum((tok_gate[e] ** 2).sum() for e in range(E)))
    tail2 = [np.cumsum(tok_gate[e][::-1].astype(np.float64) ** 2)
             for e in range(E)]  # tail2[e][k-1] = sum of k smallest gates^2

    def share_of(nbs):
        s = 0.0
        for el in range(E_LOC):
            for e in slot_exp[el]:
                k = len(tok_gate[e]) - nbs[el]
                if k > 0:
                    s += tail2[e][k - 1]
        return s / g2tot

    def nbs_for(nbtot):
        d = int(np.mean([sizes[e] for e in slot_exp[0]])
                - np.mean([sizes[e] for e in slot_exp[1]]))
        nb0 = min((nbtot + d) // 2, min(sizes[e] for e in slot_exp[0]))
        nb1 = min(nbtot - nb0, min(sizes[e] for e in slot_exp[1]))
        nb0 = nbtot - nb1
        return [nb0 & ~1, nb1 & ~1]

    lo, hi = 2 * E_LOC, int(min(sizes[e] for e in slot_exp[0])
                            + min(sizes[e] for e in slot_exp[1]))
    while lo < hi:  # find max NBtot with share <= cap (share grows as NB drops)
        mid = (lo + hi) // 2
        if share_of(nbs_for(mid)) <= SHARE_CAP:
            hi = mid
        else:
            lo = mid + 1
    NBs = nbs_for(lo)
    # round NB up so the slot's max fp8 width lands on a multiple of 16
    # (no zero-pad columns in the fp8 chunk); raising NB only lowers share
    for el in range(E_LOC):
        mx = max(sizes[e] for e in slot_exp[el])
        adj = (mx - NBs[el]) % 16
        cap = min(sizes[e] for e in slot_exp[el])
        if adj and NBs[el] + adj <= cap and (NBs[el] + adj) % 2 == 0:
            NBs[el] += adj
    nb = [0] * E
    for el in range(E_LOC):
        for e in slot_exp[el]:
            nb[e] = min(NBs[el], len(tok_gate[e]))
    nf = [len(tok_gate[e]) - nb[e] for e in range(E)]

    CBs, CFs = [], []
    for el in range(E_LOC):
        CBs.append(_even(max(nb[assign[c][el]] for c in range(N_CORES))))
        cf = _even(max(nf[assign[c][el]] for c in range(N_CORES)), lo=16)
        CFs.append(-(-cf // 16) * 16)
    assert max(CFs) <= 512, CFs
    CBT, CFT = sum(CBs), sum(CFs)
    offb = [0, CBs[0]]
    offf = [0, CFs[0]]
    hoff = [0, CBs[0] + CFs[0]]

    SX = float(224.0 / np.abs(hs).max())

    # ---- build per-core inputs ----
    in_maps = []
    for c in range(N_CORES):
        xt = np.zeros((NDT, 128, CBT), dtype=NPBF)
        x8 = np.zeros((NDT, 128, CFT), dtype=NP8)
        wv1 = np.empty((E_LOC, NIT, 128, 2 * NDT * 128), dtype=NPBF)
        wv8 = np.empty((E_LOC, NIT, 128, 2 * NDT * 128), dtype=NP8)
        w2 = np.empty((E_LOC, NDT, 128, NIT * 128), dtype=NPBF)
        scl = np.empty((128, E_LOC, 2), dtype=np.float32)
        for el in range(E_LOC):
            e = assign[c][el]
            nbe, nfe = nb[e], nf[e]
            xb = hs[tok_idx[e][:nbe]]                    # [nbe, D]
            xf = hs[tok_idx[e][nbe:]]                    # [nfe, D]
            xt[:, :, offb[el]:offb[el] + nbe] = \
                xb.T.reshape(NDT, 128, -1).astype(NPBF)
            x8[:, :, offf[el]:offf[el] + nfe] = np.clip(
                xf.T.reshape(NDT, 128, -1) * SX, -240, 240).astype(NP8)
            w1 = ws[e, :I, :]
            v1 = ws[e, I:, :]
            sw = float(224.0 / np.abs(w1).max())
            sv = float(224.0 / np.abs(v1).max())
            scl[:, el, 0] = 1.0 / (sw * SX)
            scl[:, el, 1] = 1.0 / (sv * SX)
            # wv1[el, it, p, (w, dt, ii)] = {w1,v1}[it*128+ii, dt*128+p]
            wv = np.stack([w1, v1]).reshape(2, NIT, 128, NDT, 128)
            wv = wv.transpose(1, 4, 0, 3, 2)             # [it, p, w, dt, ii]
            wv1[el] = np.ascontiguousarray(wv).reshape(
                NIT, 128, -1).astype(NPBF)
            # wv8[el, it, p, (w, u, s, ii)] = {w1*sw, v1*sv}[it*128+ii,
            #                                               (2u+s)*128+p]
            wq = np.stack([np.clip(w1 * sw, -240, 240),
                           np.clip(v1 * sv, -240, 240)])
            wq = wq.reshape(2, NIT, 128, NDP, 2, 128)    # [w,it,ii,u,s,p]
            wq = wq.transpose(1, 5, 0, 3, 4, 2)          # [it,p,w,u,s,ii]
            wv8[el] = np.ascontiguousarray(wq).reshape(
                NIT, 128, -1).astype(NP8)
            # w2[el, dt, p, (it, ii)] = w2s[e, dt*128+ii, it*128+p]
            w2e = w2s[e].reshape(NDT, 128, NIT, 128)     # [dt, ii, it, p]
            w2[el] = np.ascontiguousarray(
                w2e.transpose(0, 3, 2, 1)).reshape(NDT, 128, -1).astype(NPBF)
        in_maps.append({"xt": xt, "x8": x8, "wv1": wv1, "wv8": wv8,
                        "w2": w2, "scl": scl})

    def combine(results):
        out = np.zeros((T, D), dtype=np.float32)
        for c in range(N_CORES):
            yt = results[c]["yt"]                        # [NDT, 128, CT]
            yf = yt.reshape(D, CBT + CFT).astype(np.float32)
            for el in range(E_LOC):
                e = assign[c][el]
                n_e = len(tok_idx[e])
                if n_e == 0:
                    continue
                nbe = nb[e]
                y = np.empty((n_e, D), dtype=np.float32)
                y[:nbe] = yf[:, hoff[el]:hoff[el] + nbe].T
                y[nbe:] = yf[:, hoff[el] + CBs[el]:
                             hoff[el] + CBs[el] + (n_e - nbe)].T
                out[tok_idx[e]] += \
                    tok_gate[e][:, None].astype(np.float32) * y
        return out

    nc = _build_nc(CBs, CFs, rep=rep)
    return {"nc": nc, "in_maps": in_maps, "combine": combine,
            "C": (CBs, CFs)}


def kernel(hidden_states, router_w, ws, w2s):
    from concourse.bass_utils import run_bass_kernel_spmd
    prep = _prepare(hidden_states, router_w, ws, w2s)
    res = run_bass_kernel_spmd(prep["nc"], prep["in_maps"],
                               core_ids=list(range(N_CORES)))
    return prep["combine"](res.results)


# revision 16
# speedup vs baseline: 1.1794x; 1.0051x over previous
"""DBRX MoE experts kernel for 8 Trainium2 NeuronCores (expert-parallel).

v4: mixed-precision. Each expert's tokens are split by gate weight:
high-gate tokens run the gate/up matmuls (stage 1+2) in bf16 as before;
low-gate tokens (gate <= THETA, ~26% of columns carrying ~6% of the
squared gate mass) run stage 1+2 in fp8-e4m3 with DoubleRow matmuls
(256-deep contraction pairs, 2 MACs/cell/cycle). h stays bf16 (at a
folded scale) so stage 3 is plain bf16 for every column, and the final
copy de-scales the fp8 columns. Simulated end-to-end rel-err 1.55e-2
vs the 2e-2 gate.

  - Host: router matmul + softmax + top-4 + renormalize, gather tokens
    per expert, split by gate, pre-transpose/re-tile operands (bf16 +
    scaled fp8 copies of w1/v1/x for the low-gate groups).
  - Device (SPMD, 8 cores, 2 experts each), per expert:
       bf16 cols: GT/UT = W1T.T@XT / V1T.T@XT   (PSUM f32, NDT matmuls)
       fp8  cols: GT/UT via DoubleRow pairs      (NDT/2 matmuls)
       HT = silu(GT)*UT -> bf16 (fp8 cols keep a sv*SX scale folded in)
       YT[d] = W2T.T@HT  (bf16, all cols; fp8 cols de-scaled in the copy)
  - Host: scale rows by gates and scatter-add into the output.
"""
import sys
sys.path.insert(0, "/opt/trn_rl_repo")
import numpy as np
import ml_dtypes

import concourse.bass as bass
import concourse.mybir as mybir
import concourse.tile as tile
import concourse.tile_sem_assignment as _tsa

# Four HWDGE lanes (round-robin): weights, tokens and outputs spread
# across queues so startup fills and the output drain run in parallel.
_tsa.NUM_HWDGE_SEMS = 4

N_CORES = 8
E = 16
E_LOC = 2
D = 2048
I = 4096
TOP_K = 4
NDT = D // 128   # 16 d-tiles
NDP = NDT // 2   # 8 d-tile pairs (DoubleRow)
NIT = I // 128   # 32 i-tiles

# Cap on the fraction of total squared gate mass routed through the fp8
# path. HW-measured end-to-end rel err (gate is 2e-2): 0.078 -> 1.57e-2,
# 0.088 -> 1.66e-2, 0.098 -> 1.69e-2; error scales ~sqrt(share) above
# the bf16 floor, so 0.105 predicts ~1.74e-2.
SHARE_CAP = 0.105

BF16 = mybir.dt.bfloat16
F32 = mybir.dt.float32
FP8 = mybir.dt.float8e4
NPBF = ml_dtypes.bfloat16
NP8 = ml_dtypes.float8_e4m3
DR = mybir.MatmulPerfMode.DoubleRow


def _split_multi_waits(nc):
    """Split multi-wait instructions into single-wait EventSemaphore
    prefixes (this walrus build supports one sync-wait per instruction)."""
    ctr = 0
    for f in nc.m.functions:
        for blk in f.blocks:
            insts = list(blk.instructions)
            out = []
            changed = False
            for inst in insts:
                si = inst.sync_info
                if si is not None and si.on_wait is not None and len(si.on_wait) > 1:
                    waits = list(si.on_wait)
                    for w in waits[:-1]:
                        ctr += 1
                        out.append(mybir.InstEventSemaphore(
                            name=f"wsplit_{ctr}",
                            engine=inst.engine,
                            ins=[], outs=[],
                            sync_info=mybir.SyncInfo(on_wait=[w], on_update=[]),
                            bass_nofuse=True,
                        ))
                    inst.sync_info = mybir.SyncInfo(
                        on_wait=[waits[-1]], on_update=list(si.on_update or []))
                    changed = True
                out.append(inst)
            if changed:
                blk.instructions.clear()
                for i2 in out:
                    blk.add_instruction(i2)


def _chunks(n):
    """Split even-length [0, n) into even-sized PSUM chunks (<=512 each)."""
    assert n % 2 == 0
    if n <= 512:
        return [(0, n)]
    k = -(-n // 512)
    sizes = [(n // k) & ~1] * k
    rem, j = n - sum(sizes), 0
    while rem > 0:
        sizes[j] += 2
        rem -= 2
        j = (j + 1) % k
    out, s = [], 0
    for sz in sizes:
        out.append((s, sz))
        s += sz
    return out


def _build_nc(CBs, CFs, rep=1):
    """CBs/CFs: per-slot bf16/fp8 token widths (same for all cores)."""
    nc = bass.Bass(target_bir_lowering=False)
    CBT = sum(CBs)
    CFT = sum(CFs)
    CT = CBT + CFT
    xt_d = nc.dram_tensor("xt", [NDT, 128, CBT], BF16, kind="ExternalInput")
    x8_d = nc.dram_tensor("x8", [NDT, 128, CFT], FP8, kind="ExternalInput")
    wv1_d = nc.dram_tensor("wv1", [E_LOC, NIT, 128, 2 * NDT * 128], BF16,
                           kind="ExternalInput")
    # fp8 stationary pairs: per (el, it, partition): (w, dpair, slot, i)
    wv8_d = nc.dram_tensor("wv8", [E_LOC, NIT, 128, 2 * NDT * 128], FP8,
                           kind="ExternalInput")
    w2_d = nc.dram_tensor("w2", [E_LOC, NDT, 128, NIT * 128], BF16,
                          kind="ExternalInput")
    scl_d = nc.dram_tensor("scl", [128, E_LOC, 2], F32, kind="ExternalInput")
    yt_d = nc.dram_tensor("yt", [NDT, 128, CT], BF16, kind="ExternalOutput")

    with tile.TileContext(nc) as tc:
        with (
            tc.tile_pool(name="xt", bufs=2) as xt_pool,
            tc.tile_pool(name="ht", bufs=1) as ht_pool,
            tc.tile_pool(name="wg", bufs=3) as wg_pool,
            tc.tile_pool(name="ev", bufs=2) as ev_pool,
            tc.tile_pool(name="ps", bufs=2, space="PSUM") as ps,
        ):
            scl = ev_pool.tile([128, E_LOC, 2], F32, tag="scl", name="scl",
                               bufs=1)
            nc.sync.dma_start(scl[:], scl_d[:])
            for rp in range(rep):
                offb = 0
                offf = 0
                for el in range(E_LOC):
                    _emit_expert(nc, xt_pool, ht_pool, wg_pool, ev_pool, ps,
                                 xt_d, x8_d, wv1_d, wv8_d, w2_d, yt_d, scl,
                                 rp, el, CBs, CFs, offb, offf)
                    offb += CBs[el]
                    offf += CFs[el]
    nc.finalize()
    _split_multi_waits(nc)
    return nc


def _emit_expert(nc, xt_pool, ht_pool, wg_pool, ev_pool, ps,
                 xt_d, x8_d, wv1_d, wv8_d, w2_d, yt_d, scl,
                 rp, el, CBs, CFs, offb, offf):
    CB, CF = CBs[el], CFs[el]
    CBT, CFT = sum(CBs), sum(CFs)
    C = CB + CF
    # hts / yt column layout: [b0 | f0 | b1 | f1]
    hoff = (offb + offf)
    ch = _chunks(CB)
    tb = f"{rp}_{el}"
    # --- token tiles (resident for the whole expert) ---
    wv0 = wg_pool.tile([128, 2, NDT, 128], BF16, tag="wg", name=f"wv_{tb}_0")
    wv0r = wv1_d[el, 0].rearrange("p (w t i) -> p w t i", w=2, t=NDT)
    xts = xt_pool.tile([128, NDT, CB], BF16, tag="xt", name=f"xt_{tb}")
    xtr = xt_d[:, :, offb:offb + CB]
    if el == 0 and rp == 0:
        # fine-grained first pieces: the first matmuls wait on ~320KB only
        nc.sync.dma_start(wv0[:, :, 0:2, :], wv0r[:, :, 0:2, :])
        nc.sync.dma_start(xts[:, 0:1, :],
                          xtr[0:1].rearrange("t p c -> p t c"))
        nc.sync.dma_start(xts[:, 1:2, :],
                          xtr[1:2].rearrange("t p c -> p t c"))
        nc.sync.dma_start(wv0[:, :, 2:, :], wv0r[:, :, 2:, :])
    else:
        nc.sync.dma_start(wv0[:, :, 0:2, :], wv0r[:, :, 0:2, :])
        nc.sync.dma_start(xts[:, 0:2, :],
                          xtr[0:2].rearrange("t p c -> p t c"))
        nc.sync.dma_start(wv0[:, :, 2:, :], wv0r[:, :, 2:, :])
    x8s = xt_pool.tile([128, NDT, CF], FP8, tag="x8", name=f"x8_{tb}")
    nc.sync.dma_start(x8s[:],
                      x8_d[:, :, offf:offf + CF].rearrange("t p c -> p t c"))
    nc.sync.dma_start(xts[:, 2:8, :], xtr[2:8].rearrange("t p c -> p t c"))
    nc.sync.dma_start(xts[:, 8:, :], xtr[8:].rearrange("t p c -> p t c"))
    hts = ht_pool.tile([128, NIT, C], BF16, tag="ht", name=f"ht_{tb}")
    # --- stage 1+2: HT = silu(W1T.T@X) * (V1T.T@X), full I ---
    for it in range(NIT):
        if it == 0:
            wv = wv0
        else:
            wv = wg_pool.tile([128, 2, NDT, 128], BF16, tag="wg",
                              name=f"wv_{tb}_{it}")
            nc.sync.dma_start(
                wv[:], wv1_d[el, it].rearrange("p (w t i) -> p w t i",
                                               w=2, t=NDT))
        wv8 = wg_pool.tile([128, 2, NDP, 2, 128], FP8, tag="wg8",
                           name=f"wv8_{tb}_{it}")
        nc.sync.dma_start(
            wv8[:], wv8_d[el, it].rearrange("p (w u s i) -> p w u s i",
                                            w=2, u=NDP, s=2))
        # bf16 group
        gs = [ps.tile([128, cn], F32, tag=f"pq{ci}", name=f"g{ci}_{tb}_{it}")
              for ci, (c0, cn) in enumerate(ch)]
        for dt in range(NDT):
            for ci, (c0, cn) in enumerate(ch):
                nc.tensor.matmul(gs[ci][:], wv[:, 0, dt, :],
                                 xts[:, dt, c0:c0 + cn],
                                 start=(dt == 0), stop=(dt == NDT - 1))
        # fp8 group (DoubleRow pairs)
        gf = ps.tile([128, CF], F32, tag="pf", name=f"gf_{tb}_{it}")
        for u in range(NDP):
            nc.tensor.matmul(gf[:], wv8[:, 0, u], x8s[:, 2 * u:2 * u + 2, :],
                             start=(u == 0), stop=(u == NDP - 1),
                             perf_mode=DR)
        ss = []
        for ci, (c0, cn) in enumerate(ch):
            s = ev_pool.tile([128, cn], BF16, tag=f"hs{ci}",
                             name=f"hs{ci}_{tb}_{it}")
            nc.scalar.activation(s[:], gs[ci][:],
                                 mybir.ActivationFunctionType.Silu)
            ss.append(s)
        sf = ev_pool.tile([128, CF], BF16, tag="hsf", name=f"hsf_{tb}_{it}")
        nc.scalar.activation(sf[:], gf[:], mybir.ActivationFunctionType.Silu,
                             scale=scl[:, el, 0:1])
        us = [ps.tile([128, cn], F32, tag=f"pq{ci}", name=f"u{ci}_{tb}_{it}")
              for ci, (c0, cn) in enumerate(ch)]
        for dt in range(NDT):
            for ci, (c0, cn) in enumerate(ch):
                nc.tensor.matmul(us[ci][:], wv[:, 1, dt, :],
                                 xts[:, dt, c0:c0 + cn],
                                 start=(dt == 0), stop=(dt == NDT - 1))
        uf = ps.tile([128, CF], F32, tag="pf", name=f"uf_{tb}_{it}")
        for u in range(NDP):
            nc.tensor.matmul(uf[:], wv8[:, 1, u], x8s[:, 2 * u:2 * u + 2, :],
                             start=(u == 0), stop=(u == NDP - 1),
                             perf_mode=DR)
        for ci, (c0, cn) in enumerate(ch):
            nc.vector.tensor_tensor(
                out=hts[:, it, c0:c0 + cn], in0=us[ci][:],
                in1=ss[ci][:], op=mybir.AluOpType.mult)
        nc.vector.tensor_tensor(
            out=hts[:, it, CB:CB + CF], in0=uf[:],
            in1=sf[:], op=mybir.AluOpType.mult)
    # --- stage 3: YT[dt] = W2T.T @ HT (contract all of I), all cols bf16 ---
    ch3 = ch + [(CB, CF)]
    for dt in range(NDT):
        w2 = wg_pool.tile([128, NIT, 128], BF16, tag="w2",
                          name=f"w2_{tb}_{dt}")
        nc.sync.dma_start(
            w2[:], w2_d[el, dt].rearrange("p (u i) -> p u i", u=NIT))
        ys = [ps.tile([128, cn], F32,
                      tag=("pf" if ci == len(ch3) - 1 else f"pq{ci}"),
                      name=f"y{ci}_{tb}_{dt}")
              for ci, (c0, cn) in enumerate(ch3)]
        for it in range(NIT):
            for ci, (c0, cn) in enumerate(ch3):
                nc.tensor.matmul(ys[ci][:], w2[:, it, :],
                                 hts[:, it, c0:c0 + cn],
                                 start=(it == 0), stop=(it == NIT - 1))
        yo = ev_pool.tile([128, C], BF16, tag="yo", name=f"yo_{tb}_{dt}")
        fine = (el == E_LOC - 1)
        for ci, (c0, cn) in enumerate(ch3):
            if ci == len(ch3) - 1:
                nc.scalar.activation(yo[:, c0:c0 + cn], ys[ci][:],
                                     mybir.ActivationFunctionType.Copy,
                                     scale=scl[:, el, 1:2])
            else:
                nc.scalar.activation(yo[:, c0:c0 + cn], ys[ci][:],
                                     mybir.ActivationFunctionType.Copy)
            if fine:
                nc.sync.dma_start(yt_d[dt, :, hoff + c0:hoff + c0 + cn],
                                  yo[:, c0:c0 + cn])
        if not fine:
            nc.sync.dma_start(yt_d[dt, :, hoff:hoff + C], yo[:])


def _even(n, lo=4):
    n = max(lo, int(n))
    return n + (n % 2)


def _prepare(hidden_states, router_w, ws, w2s, rep=1):
    hs = np.ascontiguousarray(hidden_states, dtype=np.float32)
    rw = np.ascontiguousarray(router_w, dtype=np.float32)
    ws = np.asarray(ws, dtype=np.float32)
    w2s = np.asarray(w2s, dtype=np.float32)
    T, D_ = hs.shape
    assert (D_, ws.shape[0], ws.shape[1], w2s.shape[1], w2s.shape[2]) == \
        (D, E, 2 * I, D, I), "kernel compiled for DBRX 16x(2048->4096) shapes"

    # ---- routing on host (softmax -> top-4 -> renormalize) ----
    logits = hs @ rw.T
    m = logits.max(axis=-1, keepdims=True)
    p = np.exp(logits - m)
    p /= p.sum(axis=-1, keepdims=True)
    topk_idx = np.argpartition(-p, TOP_K - 1, axis=-1)[:, :TOP_K]
    topk_val = np.take_along_axis(p, topk_idx, axis=-1)
    gates_w = topk_val / topk_val.sum(axis=-1, keepdims=True)

    tok_idx, tok_gate = [None] * E, [None] * E
    flat_e = topk_idx.ravel()
    flat_g = gates_w.ravel()
    flat_t = np.repeat(np.arange(T), TOP_K)
    order = np.argsort(flat_e, kind="stable")
    se, st, sg = flat_e[order], flat_t[order], flat_g[order]
    bounds = np.searchsorted(se, np.arange(E + 1))
    for e in range(E):
        ti = st[bounds[e]:bounds[e + 1]]
        tg = sg[bounds[e]:bounds[e + 1]]
        # sort tokens by gate descending: [bf16 (big gates), fp8 (small)]
        o = np.argsort(-tg, kind="stable")
        tok_idx[e] = ti[o]
        tok_gate[e] = tg[o]

    # ---- pair large experts with small ones; slot width = per-slot max ----
    sizes = np.array([len(t) for t in tok_idx])
    order = np.argsort(-sizes, kind="stable")
    assign = [[int(order[c]), int(order[2 * N_CORES - 1 - c])]
              for c in range(N_CORES)]
    slot_exp = [[assign[c][el] for c in range(N_CORES)]
                for el in range(E_LOC)]

    # ---- split: uniform bf16 count NB per slot (zero bf16 padding); an
    # expert's smallest-gate (C_e - NB) tokens take the fp8 path. NB totals
    # are set by the squared-gate-share error budget. ----
    g2tot = float(sum((tok_gate[e] ** 2).sum() for e in range(E)))
    tail2 = [np.cumsum(tok_gate[e][::-1].astype(np.float64) ** 2)
             for e in range(E)]  # tail2[e][k-1] = sum of k smallest gates^2

    def share_of(nbs):
        s = 0.0
        for el in range(E_LOC):
            for e in slot_exp[el]:
                k = len(tok_gate[e]) - nbs[el]
                if k > 0:
                    s += tail2[e][k - 1]
        return s / g2tot

    def nbs_for(nbtot):
        d = int(np.mean([sizes[e] for e in slot_exp[0]])
                - np.mean([sizes[e] for e in slot_exp[1]]))
        nb0 = min((nbtot + d) // 2, min(sizes[e] for e in slot_exp[0]))
        nb1 = min(nbtot - nb0, min(sizes[e] for e in slot_exp[1]))
        nb0 = nbtot - nb1
        return [nb0 & ~1, nb1 & ~1]

    lo, hi = 2 * E_LOC, int(min(sizes[e] for e in slot_exp[0])
                            + min(sizes[e] for e in slot_exp[1]))
    while lo < hi:  # find max NBtot with share <= cap (share grows as NB drops)
        mid = (lo + hi) // 2
        if share_of(nbs_for(mid)) <= SHARE_CAP:
            hi = mid
        else:
            lo = mid + 1
    NBs = nbs_for(lo)
    # round NB up so the slot's max fp8 width lands on a multiple of 16
    # (no zero-pad columns in the fp8 chunk); raising NB only lowers share
    for el in range(E_LOC):
        mx = max(sizes[e] for e in slot_exp[el])
        adj = (mx - NBs[el]) % 16
        cap = min(sizes[e] for e in slot_exp[el])
        if adj and NBs[el] + adj <= cap and (NBs[el] + adj) % 2 == 0:
            NBs[el] += adj
    nb = [0] * E
    for el in range(E_LOC):
        for e in slot_exp[el]:
            nb[e] = min(NBs[el], len(tok_gate[e]))
    nf = [len(tok_gate[e]) - nb[e] for e in range(E)]

    CBs, CFs = [], []
    for el in range(E_LOC):
        CBs.append(_even(max(nb[assign[c][el]] for c in range(N_CORES))))
        cf = _even(max(nf[assign[c][el]] for c in range(N_CORES)), lo=16)
        CFs.append(-(-cf // 16) * 16)
    assert max(CFs) <= 512, CFs
    CBT, CFT = sum(CBs), sum(CFs)
    offb = [0, CBs[0]]
    offf = [0, CFs[0]]
    hoff = [0, CBs[0] + CFs[0]]

    SX = float(224.0 / np.abs(hs).max())

    # ---- build per-core inputs ----
    in_maps = []
    for c in range(N_CORES):
        xt = np.zeros((NDT, 128, CBT), dtype=NPBF)
        x8 = np.zeros((NDT, 128, CFT), dtype=NP8)
        wv1 = np.empty((E_LOC, NIT, 128, 2 * NDT * 128), dtype=NPBF)
        wv8 = np.empty((E_LOC, NIT, 128, 2 * NDT * 128), dtype=NP8)
        w2 = np.empty((E_LOC, NDT, 128, NIT * 128), dtype=NPBF)
        scl = np.empty((128, E_LOC, 2), dtype=np.float32)
        for el in range(E_LOC):
            e = assign[c][el]
            nbe, nfe = nb[e], nf[e]
            xb = hs[tok_idx[e][:nbe]]                    # [nbe, D]
            xf = hs[tok_idx[e][nbe:]]                    # [nfe, D]
            xt[:, :, offb[el]:offb[el] + nbe] = \
                xb.T.reshape(NDT, 128, -1).astype(NPBF)
            x8[:, :, offf[el]:offf[el] + nfe] = np.clip(
                xf.T.reshape(NDT, 128, -1) * SX, -240, 240).astype(NP8)
            w1 = ws[e, :I, :]
            v1 = ws[e, I:, :]
            sw = float(224.0 / np.abs(w1).max())
            sv = float(224.0 / np.abs(v1).max())
            scl[:, el, 0] = 1.0 / (sw * SX)
            scl[:, el, 1] = 1.0 / (sv * SX)
            # wv1[el, it, p, (w, dt, ii)] = {w1,v1}[it*128+ii, dt*128+p]
            wv = np.stack([w1, v1]).reshape(2, NIT, 128, NDT, 128)
            wv = wv.transpose(1, 4, 0, 3, 2)             # [it, p, w, dt, ii]
            wv1[el] = np.ascontiguousarray(wv).reshape(
                NIT, 128, -1).astype(NPBF)
            # wv8[el, it, p, (w, u, s, ii)] = {w1*sw, v1*sv}[it*128+ii,
            #                                               (2u+s)*128+p]
            wq = np.stack([np.clip(w1 * sw, -240, 240),
                           np.clip(v1 * sv, -240, 240)])
            wq = wq.reshape(2, NIT, 128, NDP, 2, 128)    # [w,it,ii,u,s,p]
            wq = wq.transpose(1, 5, 0, 3, 4, 2)          # [it,p,w,u,s,ii]
            wv8[el] = np.ascontiguousarray(wq).reshape(
                NIT, 128, -1).astype(NP8)
            # w2[el, dt, p, (it, ii)] = w2s[e, dt*128+ii, it*128+p]
            w2e = w2s[e].reshape(NDT, 128, NIT, 128)     # [dt, ii, it, p]
            w2[el] = np.ascontiguousarray(
                w2e.transpose(0, 3, 2, 1)).reshape(NDT, 128, -1).astype(NPBF)
        in_maps.append({"xt": xt, "x8": x8, "wv1": wv1, "wv8": wv8,
                        "w2": w2, "scl": scl})

    def combine(results):
        out = np.zeros((T, D), dtype=np.float32)
        for c in range(N_CORES):
            yt = results[c]["yt"]                        # [NDT, 128, CT]
            yf = yt.reshape(D, CBT + CFT).astype(np.float32)
            for el in range(E_LOC):
                e = assign[c][el]
                n_e = len(tok_idx[e])
                if n_e == 0:
                    continue
                nbe = nb[e]
                y = np.empty((n_e, D), dtype=np.float32)
                y[:nbe] = yf[:, hoff[el]:hoff[el] + nbe].T
                y[nbe:] = yf[:, hoff[el] + CBs[el]:
                             hoff[el] + CBs[el] + (n_e - nbe)].T
                out[tok_idx[e]] += \
                    tok_gate[e][:, None].astype(np.float32) * y
        return out

    nc = _build_nc(CBs, CFs, rep=rep)
    return {"nc": nc, "in_maps": in_maps, "combine": combine,
            "C": (CBs, CFs)}


def kernel(hidden_states, router_w, ws, w2s):
    from concourse.bass_utils import run_bass_kernel_spmd
    prep = _prepare(hidden_states, router_w, ws, w2s)
    res = run_bass_kernel_spmd(prep["nc"], prep["in_maps"],
                               core_ids=list(range(N_CORES)))
    return prep["combine"](res.results)
